# revision 15
# baseline (speedup 1.0000x reference)
"""Multi-head attention Bass/Tile kernel for TRN2, sharded 8 ways.

Sharding: core c handles batch b = c//2 and heads half = c%2 (8 of 16 heads).
Each core computes, for its batch and its 8 heads:
  q/k/v projections -> scoresT = K @ Q^T (per head, [t, s] layout) -> exp ->
  PV matmul with a ones-column appended to V (gives row sums for free) ->
  normalize -> partial output projection against its 512 rows of Wout^T.
Host sums the two partials per batch and adds the bias.

Layout choices (all chosen so NO transposes are needed anywhere):
  xT     [D, S]  : host-pretransposed activations (d on partitions)
  wq/wk  [D, H*dk] : lhsT layout for qT/kT = W^T @ xT
  wv     [D, H*dk] : rhs layout for v = xT^T @ wv  ([t, vdim], natural)
  kT     [H*dk, S]: j on partitions -> head-pair p lives in 128-row chunk p
  qTz    zero-padded per head: scores contract K=128 at base partition 0,
         sharing the kT stationary operand between the pair's two matmuls
  scoresT[t, s]   : lhsT=kT [j,t], rhs=qTz [j,s]; softmax sum over t is
                    folded into the PV matmul via the ones column of v'.
  out    [s, o]   : lhsT=concatT [i,s], rhs=woutT [i,o]

The whole kernel is one software pipeline over units (sb, hp): the PV
matmuls of unit k-1 are interleaved t-chunk-wise into the scores loop of
unit k so the PE never queues behind an exp it is waiting on, and the ACT
engine (the bottleneck: 33.5M exps/core) is fed continuously. The v'
projection fills the PV slot of the very first unit.

HW pitfalls baked in (learned on-device):
  - no partition-shifting DVE copies (sim allows them, HW corrupts);
    the only cross-partition moves are InstReciprocal psum[64:65]->sbuf[0:1]
    (verified on HW) and gpsimd partition_broadcast
  - reciprocal_approx_fast (custom DVE op) produces garbage on HW
  - matmul free dim capped at 512; 2-bank psum tiles need bank-aligned halves
"""

from contextlib import ExitStack
from dataclasses import dataclass

import numpy as np
import ml_dtypes

import concourse.bass as bass  # noqa: F401
import concourse.tile as tile
from concourse import bacc, mybir


@dataclass
class Cfg:
    D: int = 1024      # model dim
    S: int = 2048      # sequence length (queries == keys)
    HL: int = 8        # heads per core
    DK: int = 64       # head dim
    S_BLK: int = 512   # query block (matmul free dim)
    T_BLK: int = 512   # t block in projection phase

    @property
    def DC(self):
        return self.D // 128

    @property
    def NSB(self):
        return self.S // self.S_BLK

    @property
    def TBn(self):
        return self.S // self.T_BLK

    @property
    def TCn(self):
        return self.S // 128

    @property
    def JW(self):
        return self.HL * self.DK

    @property
    def JC(self):
        return self.JW // 128

    @property
    def VW(self):
        return self.DK + 1

    @property
    def OB(self):
        return min(512, self.D)


DT_NP = {
    mybir.dt.bfloat16: ml_dtypes.bfloat16,
    mybir.dt.float32: np.float32,
    mybir.dt.float32r: np.float32,
}


def build_nc(cfg: Cfg, DT=mybir.dt.bfloat16, num_devices: int = 8):
    c = cfg
    f32 = mybir.dt.float32
    EXPDT = DT if DT == mybir.dt.bfloat16 else f32
    SCALE = 1.0 / float(np.sqrt(c.DK))
    nc = bacc.Bacc("TRN2", target_bir_lowering=False, debug=False,
                   num_devices=num_devices)

    xqT = nc.dram_tensor("xqT", [c.D, c.S], DT, kind="ExternalInput").ap()
    xkT = nc.dram_tensor("xkT", [c.D, c.S], DT, kind="ExternalInput").ap()
    xvT = nc.dram_tensor("xvT", [c.D, c.S], DT, kind="ExternalInput").ap()
    wq_d = nc.dram_tensor("wq", [c.D, c.JW], DT, kind="ExternalInput").ap()
    wk_d = nc.dram_tensor("wk", [c.D, c.JW], DT, kind="ExternalInput").ap()
    wv_d = nc.dram_tensor("wv", [c.D, c.JW], DT, kind="ExternalInput").ap()
    wo_d = nc.dram_tensor("woutT", [c.JW, c.D], DT, kind="ExternalInput").ap()
    out_d = nc.dram_tensor("out", [c.S, c.D], f32, kind="ExternalOutput").ap()

    from collections import deque

    with tile.TileContext(nc) as tc, ExitStack() as es:
        wpool = es.enter_context(tc.tile_pool(name="weights", bufs=1))
        kvpool = es.enter_context(tc.tile_pool(name="kv", bufs=1))
        xkpool = es.enter_context(tc.tile_pool(name="xk", bufs=4))
        xqpool = es.enter_context(tc.tile_pool(name="xq", bufs=3))
        xvpool = es.enter_context(tc.tile_pool(name="xv", bufs=2))
        qpool = es.enter_context(tc.tile_pool(name="q", bufs=2))
        epool = es.enter_context(tc.tile_pool(name="exp", bufs=12))
        cpool = es.enter_context(tc.tile_pool(name="cat", bufs=2))
        opool = es.enter_context(tc.tile_pool(name="o", bufs=2))
        rpool = es.enter_context(tc.tile_pool(name="r", bufs=1))
        stpool = es.enter_context(tc.tile_pool(name="st", bufs=2))
        pspool = es.enter_context(tc.tile_pool(name="ps", bufs=2, space="PSUM"))
        pvpool = es.enter_context(tc.tile_pool(name="pv", bufs=2, space="PSUM"))
        fppool = es.enter_context(tc.tile_pool(name="fp", bufs=2, space="PSUM"))

        def load_w_dmaj(dram, width, tag):
            t = wpool.tile([128, c.DC * width], DT, tag=tag, name=tag)
            for d in range(c.DC):
                eng = nc.sync if d % 2 == 0 else nc.gpsimd
                eng.dma_start(t[:, d * width:(d + 1) * width],
                              dram[d * 128:(d + 1) * 128, :])
            return t

        def load_x(pool, dram, blk, width, name):
            t = pool.tile([128, c.DC * width], DT, tag="x", name=name)
            for d in range(c.DC):
                eng = nc.sync if d % 2 == 0 else nc.gpsimd
                eng.dma_start(
                    t[:, d * width:(d + 1) * width],
                    dram[d * 128:(d + 1) * 128, blk * width:(blk + 1) * width])
            return t

        NT = c.T_BLK

        # ---- head DMAs: wk + xk lead (kT jc0 inline), then wq/xq0 for
        # qT(0) jc0, then wv/xv + wo for the v/outproj fillers.
        wk_sb = load_w_dmaj(wk_d, c.JW, "wk")
        kT_sb = kvpool.tile([128, c.JC * c.S], DT, tag="kT", name="kT")
        xk_tiles = [load_x(xkpool, xkT, tb, NT, f"xk{tb}")
                    for tb in range(c.TBn)]
        wq_sb = load_w_dmaj(wq_d, c.JW, "wq")
        xq_tiles = {0: load_x(xqpool, xqT, 0, c.S_BLK, "xq0"),
                    1: load_x(xqpool, xqT, 1, c.S_BLK, "xq1")}
        wv_sb = load_w_dmaj(wv_d, c.JW, "wv")
        v_sb = kvpool.tile([128, c.TCn * c.HL * c.VW], DT, tag="v", name="v")
        nc.gpsimd.memset(v_sb[:], 1.0)  # ones columns preset
        xv_tiles = {0: load_x(xvpool, xvT, 0, NT, "xv0"),
                    1: load_x(xvpool, xvT, 1, NT, "xv1")}
        wo_sb = wpool.tile([128, c.JC * c.D], DT, tag="wo", name="wo")
        for ic in range(c.JC):
            nc.sync.dma_start(wo_sb[:, ic * c.D:(ic + 1) * c.D],
                              wo_d[ic * 128:(ic + 1) * 128, :])

        def kT_ops(tb, jc):
            """8 MM closures computing kT chunk jc for t-block tb."""
            box = {}

            def mk(d):
                def op():
                    if d == 0:
                        box["ps"] = pspool.tile([128, NT], f32, tag="ps",
                                                name=f"psk{tb}_{jc}")
                    nc.tensor.matmul(
                        box["ps"][:],
                        wk_sb[:, d * c.JW + jc * 128: d * c.JW + (jc + 1) * 128],
                        xk_tiles[tb][:, d * NT:(d + 1) * NT],
                        start=(d == 0), stop=(d == c.DC - 1))
                    if d == c.DC - 1:
                        nc.vector.tensor_copy(
                            kT_sb[:, jc * c.S + tb * NT: jc * c.S + (tb + 1) * NT],
                            box["ps"][:])
                return op
            return [mk(d) for d in range(c.DC)]

        def v_ops(tb):
            """v' projection closures for t-block tb (+ trailing xv prefetch —
            after the consuming MMs so the xv ring reuse sees its readers)."""
            ops = []
            for tt in range(NT // 128):
                g = tb * (NT // 128) + tt
                box = {}

                def mk(d, g=g, tt=tt, tb=tb, box=box):
                    def op():
                        if d == 0:
                            box["ps"] = pspool.tile([128, c.JW], f32, tag="ps",
                                                    name=f"psv{g}")
                        nc.tensor.matmul(
                            box["ps"][:],
                            xv_tiles[tb][:, d * NT + tt * 128:
                                         d * NT + (tt + 1) * 128],
                            wv_sb[:, d * c.JW:(d + 1) * c.JW],
                            start=(d == 0), stop=(d == c.DC - 1))
                        if d == c.DC - 1:
                            dst = v_sb[:, g * c.HL * c.VW:(g + 1) * c.HL * c.VW]
                            dst3 = dst.rearrange("p (h w) -> p h w",
                                                 w=c.VW)[:, :, 0:c.DK]
                            src3 = box["ps"][:].rearrange("p (h w) -> p h w",
                                                          w=c.DK)
                            nc.vector.tensor_copy(dst3, src3)
                    return op
                ops += [mk(d) for d in range(c.DC)]
            if tb + 2 < c.TBn:
                def pf(tb=tb):
                    xv_tiles[tb + 2] = load_x(xvpool, xvT, tb + 2, NT,
                                              f"xv{tb + 2}")
                ops.append(pf)
            return ops

        def emit_qT_mms(sb, xq, qT):
            """32 MM closures (jc-major); last per jc copies psum -> qT chunk
            jc (head A rows 0:64, head B rows 64:128 — natural layout)."""
            ops = []
            psq_box = {}

            def mk(jc, d):
                def op():
                    if d == 0:
                        psq_box[jc] = fppool.tile([128, c.S_BLK], f32, tag="fp",
                                                  name=f"psq{sb}_{jc}")
                    nc.tensor.matmul(
                        psq_box[jc][:],
                        wq_sb[:, d * c.JW + jc * 128: d * c.JW + (jc + 1) * 128],
                        xq[:, d * c.S_BLK:(d + 1) * c.S_BLK],
                        start=(d == 0), stop=(d == c.DC - 1))
                    if d == c.DC - 1:
                        nc.vector.tensor_copy(
                            qT[:, jc * c.S_BLK:(jc + 1) * c.S_BLK],
                            psq_box[jc][:])
                return op
            for jc in range(c.JC):
                for d in range(c.DC):
                    ops.append(mk(jc, d))
            return ops

        def emit_outproj_mms(sb, catT):
            """Closures: per (sc, oc): 4 ic-MMs into a 1-bank psum, then
            copy + DMA out."""
            ops = []
            po_box = {}

            def mk(sc, oc, ic):
                def op():
                    if ic == 0:
                        po_box[(sc, oc)] = fppool.tile(
                            [128, c.OB], f32, tag="fp", name=f"po{sb}_{sc}_{oc}")
                    po = po_box[(sc, oc)]
                    nc.tensor.matmul(
                        po[:],
                        catT[:, ic * c.S_BLK + sc * 128:
                             ic * c.S_BLK + (sc + 1) * 128],
                        wo_sb[:, ic * c.D + oc * c.OB:
                              ic * c.D + (oc + 1) * c.OB],
                        start=(ic == 0), stop=(ic == c.JC - 1))
                    if ic == c.JC - 1:
                        ot = opool.tile([128, c.OB], f32, tag="ot",
                                        name=f"ot{sb}_{sc}_{oc}")
                        nc.vector.tensor_copy(ot[:], po[:])
                        nc.sync.dma_start(
                            out_d[sb * c.S_BLK + sc * 128:
                                  sb * c.S_BLK + (sc + 1) * 128,
                                  oc * c.OB:(oc + 1) * c.OB],
                            ot[:])
                return op
            for sc in range(c.S_BLK // 128):
                for oc in range(c.D // c.OB):
                    for ic in range(c.JC):
                        ops.append(mk(sc, oc, ic))
            return ops

        # ---- inline head compute: kT jc0 (all tb) + qT(0) jc0 ----
        for tb in range(c.TBn):
            for op in kT_ops(tb, 0):
                op()
        qT_tiles = {0: qpool.tile([128, c.JC * c.S_BLK], DT, tag="qT",
                                  name="qT0")}
        q0 = emit_qT_mms(0, xq_tiles[0], qT_tiles[0])
        for op in q0[0:8]:
            op()

        # ---- units + filler lists ----
        units = [(sb, hp) for sb in range(c.NSB) for hp in range(c.JC)]
        fillers = [[] for _ in units]
        # prologue fill: v rides early (PV of unit (0,0) consumes it with a
        # lag); kT jc1/jc2/jc3 + qT(0) jc1/jc2/jc3 ahead of their units.
        for _op in v_ops(0) + v_ops(1) + v_ops(2) + v_ops(3):
            _op()
        fillers[0] += kT_ops(0, 1) + kT_ops(1, 1) \
            + kT_ops(2, 1) + kT_ops(3, 1) + q0[8:16]
        fillers[1] += kT_ops(0, 2) + kT_ops(1, 2) \
            + kT_ops(2, 2) + kT_ops(3, 2) + q0[16:24]
        fillers[2] += kT_ops(0, 3) + kT_ops(1, 3) + kT_ops(2, 3) \
            + kT_ops(3, 3) + q0[24:32]

        cat_tiles = {}

        # ---- lagged-PV queue machinery ----
        pvq = deque()        # (key, op, islast)
        stage_runs = {}      # key -> closure(cur_idx)
        PVLAG = 6            # max pending pv ops before forced pops

        def emit_stage_normalize(sb, hp, catT, pv_state, cur_idx):
            stA = stpool.tile([c.VW, c.S_BLK], f32, tag="stA",
                              name=f"stA{sb}_{hp}")
            stB = stpool.tile([c.VW, c.S_BLK], f32, tag="stB",
                              name=f"stB{sb}_{hp}")
            nc.vector.tensor_copy(stA[:], pv_state["pvA"][0:c.VW, :])
            nc.vector.tensor_copy(stB[:], pv_state["pvB"][0:c.VW, :])
            rtiA = rpool.tile([1, c.S_BLK], f32, tag="rtiA",
                              name=f"rtiA{sb}_{hp}")
            rtiB = rpool.tile([1, c.S_BLK], f32, tag="rtiB",
                              name=f"rtiB{sb}_{hp}")
            # cross-partition (row 64 -> row 0) — verified OK on HW for
            # InstReciprocal specifically.
            nc.vector.reciprocal(rtiA[:], stA[c.DK:c.DK + 1, :])
            nc.vector.reciprocal(rtiB[:], stB[c.DK:c.DK + 1, :])
            rbA = rpool.tile([c.DK, c.S_BLK], f32, tag="rbA",
                             name=f"rbA{sb}_{hp}")
            rbB = rpool.tile([c.DK, c.S_BLK], f32, tag="rbB",
                             name=f"rbB{sb}_{hp}")
            nc.gpsimd.partition_broadcast(rbA[:], rtiA[:])
            nc.gpsimd.partition_broadcast(rbB[:], rtiB[:])
            nc.vector.tensor_mul(
                catT[0:c.DK, hp * c.S_BLK:(hp + 1) * c.S_BLK],
                stA[0:c.DK, :], rbA[:])
            nc.vector.tensor_mul(
                catT[64:64 + c.DK, hp * c.S_BLK:(hp + 1) * c.S_BLK],
                stB[0:c.DK, :], rbB[:])
            if hp == c.JC - 1:
                oops = emit_outproj_mms(sb, catT)
                splits = [(0, 12), (12, 22), (22, 32)]
                for j, (lo, hi) in enumerate(splits):
                    tgt = cur_idx + 1 + j
                    if tgt < len(units):
                        fillers[min(tgt, len(units) - 1)] += oops[lo:hi]
                    else:
                        for op in oops[lo:hi]:
                            op()

        def make_pv_ops(key, sb, hp, t, es_tile, pv_state):
            W = c.HL * c.VW

            def opA():
                if t == 0:
                    pv_state["pvA"] = pvpool.tile([128, c.S_BLK], f32,
                                                  tag="pv", name=f"pvA{sb}_{hp}")
                nc.tensor.matmul(
                    pv_state["pvA"][0:c.VW, :],
                    v_sb[:, t * W + (2 * hp) * c.VW:
                         t * W + (2 * hp + 1) * c.VW],
                    es_tile[:, 0:c.S_BLK],
                    start=(t == 0), stop=(t == c.TCn - 1))

            def opB():
                if t == 0:
                    pv_state["pvB"] = pvpool.tile([128, c.S_BLK], f32,
                                                  tag="pv", name=f"pvB{sb}_{hp}")
                nc.tensor.matmul(
                    pv_state["pvB"][0:c.VW, :],
                    v_sb[:, t * W + (2 * hp + 1) * c.VW:
                         t * W + (2 * hp + 2) * c.VW],
                    es_tile[:, c.S_BLK:2 * c.S_BLK],
                    start=(t == 0), stop=(t == c.TCn - 1))
            return [(key, opA, False), (key, opB, t == c.TCn - 1)]

        def pop_pv(n, cur_idx):
            for _ in range(n):
                if not pvq:
                    return
                key, op, islast = pvq.popleft()
                op()
                if islast:
                    stage_runs.pop(key)(cur_idx)

        # ---- main pipeline over units ----
        for idx, (sb, hp) in enumerate(units):
            if hp == 0:
                cat_tiles[sb] = cpool.tile([128, c.JC * c.S_BLK], DT,
                                           tag="cat", name=f"catT{sb}")
            if sb == 0 and hp == 2 and c.NSB > 1:
                # qT(1) fillers late in sb 0 (xq1 was head-loaded); xq2 load
                # goes to a fresh ring slot (xqpool bufs=3)
                if c.NSB > 2:
                    xq_tiles[2] = load_x(xqpool, xqT, 2, c.S_BLK, "xq2")
                qT_tiles[1] = qpool.tile([128, c.JC * c.S_BLK], DT,
                                         tag="qT", name="qT1")
                q1 = emit_qT_mms(1, xq_tiles[1], qT_tiles[1])
                fillers[idx] += q1[:16]
                fillers[min(idx + 1, len(units) - 1)] += q1[16:]
            if sb >= 1 and hp == 0 and sb + 1 < c.NSB:
                # steady state: xq(sb+2) prefetch + qT(sb+1) fillers spread
                # over all four units of this sb
                if sb + 2 < c.NSB and sb + 2 not in xq_tiles:
                    def pfq(sb=sb):
                        xq_tiles[sb + 2] = load_x(xqpool, xqT, sb + 2,
                                                  c.S_BLK, f"xq{sb + 2}")
                    fillers[idx].append(pfq)
                qT_tiles[sb + 1] = qpool.tile([128, c.JC * c.S_BLK], DT,
                                              tag="qT", name=f"qT{sb + 1}")
                qops = emit_qT_mms(sb + 1, xq_tiles[sb + 1], qT_tiles[sb + 1])
                for j in range(4):
                    fillers[min(idx + j, len(units) - 1)] += qops[j * 8:(j + 1) * 8]
            catT = cat_tiles[sb]
            qT = qT_tiles[sb]
            key = (sb, hp)
            pv_state = {}
            stage_runs[key] = (
                lambda cur_idx, sb=sb, hp=hp, catT=catT, pv_state=pv_state:
                emit_stage_normalize(sb, hp, catT, pv_state, cur_idx))
            flist = fillers[idx]
            fpos = 0
            for th in range(c.TCn // 2):
                for u in range(2):
                    t = 2 * th + u
                    kcol = slice(hp * c.S + t * 128, hp * c.S + (t + 1) * 128)
                    qcol = slice(hp * c.S_BLK, (hp + 1) * c.S_BLK)
                    # One 2-bank psum tile [A(512) | B(512)] per t-chunk:
                    # K=64 row-tiled pair (head A rows 0-63 tile (0,0), head B
                    # rows 64-127 tile (64,0)) in different banks; ONE exp
                    # covers both heads so the pair stays adjacent/concurrent.
                    ps2 = pspool.tile([128, 2 * c.S_BLK], f32, tag="ps",
                                      name=f"ps2_{sb}_{hp}_{t}")
                    nc.tensor.matmul(
                        ps2[:, 0:c.S_BLK],
                        kT_sb[0:64, kcol], qT[0:64, qcol],
                        start=True, stop=True)
                    nc.tensor.matmul(
                        ps2[:, c.S_BLK:2 * c.S_BLK],
                        kT_sb[64:128, kcol], qT[64:128, qcol],
                        start=True, stop=True)
                    es_t = epool.tile([128, 2 * c.S_BLK], EXPDT, tag="exp",
                                      name=f"es{sb}_{hp}_{t}")
                    nc.scalar.activation(
                        es_t[:], ps2[:], mybir.ActivationFunctionType.Exp,
                        scale=SCALE)
                    pvq.extend(make_pv_ops(key, sb, hp, t, es_t, pv_state))
                    pop_pv(len(pvq) - PVLAG, idx)
                want = (len(flist) * (th + 1)) // (c.TCn // 2)
                while fpos < want:
                    flist[fpos]()
                    fpos += 1
            # any fillers appended after pacing window closed
            while fpos < len(flist):
                flist[fpos]()
                fpos += 1
        # ---- drain ----
        pop_pv(len(pvq), len(units) - 1)

    nc.compile()
    return nc


def shard_inputs(inputs: dict, cfg: Cfg, DT=mybir.dt.bfloat16):
    """Full inputs -> list of 8 per-core in_maps (numpy)."""
    npdt = DT_NP[DT]
    q, k, v = inputs["queries"], inputs["keys"], inputs["values"]
    Wq, Wk, Wv = inputs["Wq"], inputs["Wk"], inputs["Wv"]
    Wout = inputs["Wout"]
    B = q.shape[0]
    maps = []
    WoutT = np.ascontiguousarray(Wout.T)  # [i, o]
    for core in range(2 * B):
        b, half = divmod(core, 2)
        hs = slice(half * cfg.HL, (half + 1) * cfg.HL)
        i0 = half * cfg.JW
        maps.append({
            "xqT": np.ascontiguousarray(q[b].T).astype(npdt),
            "xkT": np.ascontiguousarray(k[b].T).astype(npdt),
            "xvT": np.ascontiguousarray(v[b].T).astype(npdt),
            "wq": np.ascontiguousarray(
                Wq[hs].transpose(1, 0, 2).reshape(cfg.D, cfg.JW)).astype(npdt),
            "wk": np.ascontiguousarray(
                Wk[hs].transpose(1, 0, 2).reshape(cfg.D, cfg.JW)).astype(npdt),
            "wv": np.ascontiguousarray(
                Wv[hs].transpose(1, 0, 2).reshape(cfg.D, cfg.JW)).astype(npdt),
            "woutT": np.ascontiguousarray(WoutT[i0:i0 + cfg.JW]).astype(npdt),
        })
    return maps


def gather_outputs(results, inputs):
    bout = inputs["bout"]
    B = inputs["queries"].shape[0]
    outs = []
    for b in range(B):
        outs.append(results[2 * b]["out"] + results[2 * b + 1]["out"] + bout)
    return np.stack(outs).astype(np.float32)


def percore_reference(in_map: dict, cfg: Cfg):
    """Numpy reference of what one core should produce (fp32 math)."""
    c = cfg
    xq = in_map["xqT"].astype(np.float32).T   # [S, D]
    xk = in_map["xkT"].astype(np.float32).T
    xv = in_map["xvT"].astype(np.float32).T
    wq = in_map["wq"].astype(np.float32)      # [D, JW]
    wk = in_map["wk"].astype(np.float32)
    wv = in_map["wv"].astype(np.float32)
    wo = in_map["woutT"].astype(np.float32)   # [JW, D]
    q = xq @ wq                               # [S, JW]
    k = xk @ wk
    v = xv @ wv
    cat = np.zeros((c.S, c.JW), dtype=np.float32)
    for h in range(c.HL):
        sl = slice(h * c.DK, (h + 1) * c.DK)
        s = (q[:, sl] @ k[:, sl].T) / np.sqrt(c.DK)
        e = np.exp(s)
        p = e / e.sum(axis=1, keepdims=True)
        cat[:, sl] = p @ v[:, sl]
    return cat @ wo

# ----------------------------------------------------------------------------
# Self-contained entry point: kernel(**inputs) -> full [B, S, D] output.
# ----------------------------------------------------------------------------
_NC_CACHE = {}


def _get_nc():
    key = "attn"
    if key not in _NC_CACHE:
        _NC_CACHE[key] = build_nc(Cfg(), mybir.dt.bfloat16, num_devices=8)
    return _NC_CACHE[key]


def kernel(**inputs):
    """Full (unsharded) inputs -> full [4, 2048, 1024] float32 output.

    Shards across the 8 NeuronCores as (batch x head-half), runs the Bass
    kernel SPMD, and gathers: out[b] = partial(core 2b) + partial(core 2b+1)
    + bias (row-sharded fc_out -> partial-sum reduction at gather time).
    """
    from concourse.bass_utils import run_bass_kernel_spmd

    inputs = {k: np.asarray(v) for k, v in inputs.items()}
    cfg = Cfg()
    nc = _get_nc()
    maps = shard_inputs(inputs, cfg, mybir.dt.bfloat16)
    res = run_bass_kernel_spmd(nc, maps, core_ids=list(range(8)), trace=False)
    return gather_outputs(res.results, inputs)



# revision 29
# speedup vs baseline: 1.0642x; 1.0642x over previous
"""Multi-head attention Bass/Tile kernel for TRN2, sharded 8 ways.

Sharding: core c handles batch b = c//2 and heads half = c%2 (8 of 16 heads).
Each core computes, for its batch and its 8 heads:
  q/k/v projections -> scoresT = K @ Q^T (per head, [t, s] layout) -> exp ->
  PV matmul with a ones-column appended to V (gives row sums for free) ->
  normalize -> partial output projection against its 512 rows of Wout^T.
Host sums the two partials per batch and adds the bias.

Layout choices (all chosen so NO transposes are needed anywhere):
  xT     [D, S]  : host-pretransposed activations (d on partitions)
  wq/wk  [D, H*dk] : lhsT layout for qT/kT = W^T @ xT
  wv     [D, H*dk] : rhs layout for v = xT^T @ wv  ([t, vdim], natural)
  kT     [H*dk, S]: j on partitions -> head-pair p lives in 128-row chunk p
  qTz    zero-padded per head: scores contract K=128 at base partition 0,
         sharing the kT stationary operand between the pair's two matmuls
  scoresT[t, s]   : lhsT=kT [j,t], rhs=qTz [j,s]; softmax sum over t is
                    folded into the PV matmul via the ones column of v'.
  out    [s, o]   : lhsT=concatT [i,s], rhs=woutT [i,o]

The whole kernel is one software pipeline over units (sb, hp): the PV
matmuls of unit k-1 are interleaved t-chunk-wise into the scores loop of
unit k so the PE never queues behind an exp it is waiting on, and the ACT
engine (the bottleneck: 33.5M exps/core) is fed continuously. The v'
projection fills the PV slot of the very first unit.

HW pitfalls baked in (learned on-device):
  - no partition-shifting DVE copies (sim allows them, HW corrupts);
    the only cross-partition moves are InstReciprocal psum[64:65]->sbuf[0:1]
    (verified on HW) and gpsimd partition_broadcast
  - reciprocal_approx_fast (custom DVE op) produces garbage on HW
  - matmul free dim capped at 512; 2-bank psum tiles need bank-aligned halves
"""

from contextlib import ExitStack
from dataclasses import dataclass

import numpy as np
import ml_dtypes

import concourse.bass as bass  # noqa: F401
import concourse.tile as tile
from concourse import bacc, mybir


@dataclass
class Cfg:
    D: int = 1024      # model dim
    S: int = 2048      # sequence length (queries == keys)
    HL: int = 8        # heads per core
    DK: int = 64       # head dim
    S_BLK: int = 512   # query block (matmul free dim)
    T_BLK: int = 512   # t block in projection phase

    @property
    def DC(self):
        return self.D // 128

    @property
    def NSB(self):
        return self.S // self.S_BLK

    @property
    def TBn(self):
        return self.S // self.T_BLK

    @property
    def TCn(self):
        return self.S // 128

    @property
    def JW(self):
        return self.HL * self.DK

    @property
    def JC(self):
        return self.JW // 128

    @property
    def VW(self):
        return self.DK + 1

    @property
    def OB(self):
        return min(512, self.D)


DT_NP = {
    mybir.dt.bfloat16: ml_dtypes.bfloat16,
    mybir.dt.float32: np.float32,
    mybir.dt.float32r: np.float32,
}


def build_nc(cfg: Cfg, DT=mybir.dt.bfloat16, num_devices: int = 8):
    c = cfg
    f32 = mybir.dt.float32
    EXPDT = DT if DT == mybir.dt.bfloat16 else f32
    SCALE = 1.0 / float(np.sqrt(c.DK))
    nc = bacc.Bacc("TRN2", target_bir_lowering=False, debug=False,
                   num_devices=num_devices)

    xqT = nc.dram_tensor("xqT", [c.D, c.S], DT, kind="ExternalInput").ap()
    xkT = nc.dram_tensor("xkT", [c.D, c.S], DT, kind="ExternalInput").ap()
    xvT = nc.dram_tensor("xvT", [c.D, c.S], DT, kind="ExternalInput").ap()
    wq_d = nc.dram_tensor("wq", [c.D, c.JW], DT, kind="ExternalInput").ap()
    wk_d = nc.dram_tensor("wk", [c.D, c.JW], DT, kind="ExternalInput").ap()
    wv_d = nc.dram_tensor("wv", [c.D, c.JW], DT, kind="ExternalInput").ap()
    wo_d = nc.dram_tensor("woutT", [c.JW, c.D], DT, kind="ExternalInput").ap()
    out_d = nc.dram_tensor("out", [c.S, c.D], f32, kind="ExternalOutput").ap()

    from collections import deque

    with tile.TileContext(nc) as tc, ExitStack() as es:
        wpool = es.enter_context(tc.tile_pool(name="weights", bufs=1))
        kvpool = es.enter_context(tc.tile_pool(name="kv", bufs=1))
        xkpool = es.enter_context(tc.tile_pool(name="xk", bufs=4))
        xqpool = es.enter_context(tc.tile_pool(name="xq", bufs=3))
        xvpool = es.enter_context(tc.tile_pool(name="xv", bufs=2))
        qpool = es.enter_context(tc.tile_pool(name="q", bufs=2))
        epool = es.enter_context(tc.tile_pool(name="exp", bufs=16))
        cpool = es.enter_context(tc.tile_pool(name="cat", bufs=2))
        opool = es.enter_context(tc.tile_pool(name="o", bufs=2))
        rpool = es.enter_context(tc.tile_pool(name="r", bufs=1))
        stpool = es.enter_context(tc.tile_pool(name="st", bufs=2))
        pspool = es.enter_context(tc.tile_pool(name="ps", bufs=2, space="PSUM"))
        pvpool = es.enter_context(tc.tile_pool(name="pv", bufs=2, space="PSUM"))
        fppool = es.enter_context(tc.tile_pool(name="fp", bufs=2, space="PSUM"))

        def load_w_dmaj(dram, width, tag):
            t = wpool.tile([128, c.DC * width], DT, tag=tag, name=tag)
            for d in range(c.DC):
                eng = nc.sync if d % 2 == 0 else nc.gpsimd
                eng.dma_start(t[:, d * width:(d + 1) * width],
                              dram[d * 128:(d + 1) * 128, :])
            return t

        def load_x(pool, dram, blk, width, name):
            t = pool.tile([128, c.DC * width], DT, tag="x", name=name)
            for d in range(c.DC):
                eng = nc.sync if d % 2 == 0 else nc.gpsimd
                eng.dma_start(
                    t[:, d * width:(d + 1) * width],
                    dram[d * 128:(d + 1) * 128, blk * width:(blk + 1) * width])
            return t

        NT = c.T_BLK

        # ---- head DMAs: wk + xk lead (kT jc0 inline), then wq/xq0 for
        # qT(0) jc0, then wv/xv + wo for the v/outproj fillers.
        wk_sb = load_w_dmaj(wk_d, c.JW, "wk")
        kT_sb = kvpool.tile([128, c.JC * c.S], DT, tag="kT", name="kT")
        xk_tiles = [load_x(xkpool, xkT, tb, NT, f"xk{tb}")
                    for tb in range(c.TBn)]
        wq_sb = load_w_dmaj(wq_d, c.JW, "wq")
        xq_tiles = {0: load_x(xqpool, xqT, 0, c.S_BLK, "xq0")}
        wv_sb = load_w_dmaj(wv_d, c.JW, "wv")
        v_sb = kvpool.tile([128, c.TCn * c.HL * c.VW], DT, tag="v", name="v")
        nc.gpsimd.memset(v_sb[:], 1.0)  # ones columns preset
        xv_tiles = {0: load_x(xvpool, xvT, 0, NT, "xv0"),
                    1: load_x(xvpool, xvT, 1, NT, "xv1")}
        xq_tiles[1] = load_x(xqpool, xqT, 1, c.S_BLK, "xq1")
        wo_sb = wpool.tile([128, c.JC * c.D], DT, tag="wo", name="wo")
        for ic in range(c.JC):
            nc.sync.dma_start(wo_sb[:, ic * c.D:(ic + 1) * c.D],
                              wo_d[ic * 128:(ic + 1) * 128, :])

        def kT_ops(tb, jc):
            """8 MM closures computing kT chunk jc for t-block tb."""
            box = {}

            def mk(d):
                def op():
                    if d == 0:
                        box["ps"] = pspool.tile([128, NT], f32, tag="ps",
                                                name=f"psk{tb}_{jc}")
                    nc.tensor.matmul(
                        box["ps"][:],
                        wk_sb[:, d * c.JW + jc * 128: d * c.JW + (jc + 1) * 128],
                        xk_tiles[tb][:, d * NT:(d + 1) * NT],
                        start=(d == 0), stop=(d == c.DC - 1))
                    if d == c.DC - 1:
                        nc.vector.tensor_copy(
                            kT_sb[:, jc * c.S + tb * NT: jc * c.S + (tb + 1) * NT],
                            box["ps"][:])
                return op
            return [mk(d) for d in range(c.DC)]

        # count of v' chunk-groups whose SBUF copy has been EMITTED — PV pops
        # for sb-0 units must not overtake this (emission order defines the
        # read/write ordering the tracker enforces; a PV matmul emitted before
        # its v chunk's copy reads the memset ones instead).
        v_done = [0]

        def v_ops(tb):
            """v' projection closures for t-block tb (+ trailing xv prefetch —
            after the consuming MMs so the xv ring reuse sees its readers)."""
            ops = []
            for tt in range(NT // 128):
                g = tb * (NT // 128) + tt
                box = {}

                def mk(d, g=g, tt=tt, tb=tb, box=box):
                    def op():
                        if d == 0:
                            box["ps"] = pspool.tile([128, c.JW], f32, tag="ps",
                                                    name=f"psv{g}")
                        nc.tensor.matmul(
                            box["ps"][:],
                            xv_tiles[tb][:, d * NT + tt * 128:
                                         d * NT + (tt + 1) * 128],
                            wv_sb[:, d * c.JW:(d + 1) * c.JW],
                            start=(d == 0), stop=(d == c.DC - 1))
                        if d == c.DC - 1:
                            dst = v_sb[:, g * c.HL * c.VW:(g + 1) * c.HL * c.VW]
                            dst3 = dst.rearrange("p (h w) -> p h w",
                                                 w=c.VW)[:, :, 0:c.DK]
                            src3 = box["ps"][:].rearrange("p (h w) -> p h w",
                                                          w=c.DK)
                            nc.vector.tensor_copy(dst3, src3)
                            v_done[0] = g + 1
                    return op
                ops += [mk(d) for d in range(c.DC)]
            if tb + 2 < c.TBn:
                def pf(tb=tb):
                    xv_tiles[tb + 2] = load_x(xvpool, xvT, tb + 2, NT,
                                              f"xv{tb + 2}")
                ops.append(pf)
            return ops

        def emit_qT_mms(sb, xq, qT):
            """32 MM closures (jc-major); last per jc copies psum -> qT chunk
            jc (head A rows 0:64, head B rows 64:128 — natural layout)."""
            ops = []
            psq_box = {}

            def mk(jc, d):
                def op():
                    if d == 0:
                        psq_box[jc] = fppool.tile([128, c.S_BLK], f32, tag="fp",
                                                  name=f"psq{sb}_{jc}")
                    nc.tensor.matmul(
                        psq_box[jc][:],
                        wq_sb[:, d * c.JW + jc * 128: d * c.JW + (jc + 1) * 128],
                        xq[:, d * c.S_BLK:(d + 1) * c.S_BLK],
                        start=(d == 0), stop=(d == c.DC - 1))
                    if d == c.DC - 1:
                        nc.vector.tensor_copy(
                            qT[:, jc * c.S_BLK:(jc + 1) * c.S_BLK],
                            psq_box[jc][:])
                return op
            for jc in range(c.JC):
                for d in range(c.DC):
                    ops.append(mk(jc, d))
            return ops

        def emit_outproj_mms(sb, catT):
            """Closures: per (sc, oc): 4 ic-MMs into a 1-bank psum, then
            copy + DMA out."""
            ops = []
            po_box = {}

            def mk(sc, oc, ic):
                def op():
                    if ic == 0:
                        po_box[(sc, oc)] = fppool.tile(
                            [128, c.OB], f32, tag="fp", name=f"po{sb}_{sc}_{oc}")
                    po = po_box[(sc, oc)]
                    nc.tensor.matmul(
                        po[:],
                        catT[:, ic * c.S_BLK + sc * 128:
                             ic * c.S_BLK + (sc + 1) * 128],
                        wo_sb[:, ic * c.D + oc * c.OB:
                              ic * c.D + (oc + 1) * c.OB],
                        start=(ic == 0), stop=(ic == c.JC - 1))
                    if ic == c.JC - 1:
                        ot = opool.tile([128, c.OB], f32, tag="ot",
                                        name=f"ot{sb}_{sc}_{oc}")
                        nc.vector.tensor_copy(ot[:], po[:])
                        nc.sync.dma_start(
                            out_d[sb * c.S_BLK + sc * 128:
                                  sb * c.S_BLK + (sc + 1) * 128,
                                  oc * c.OB:(oc + 1) * c.OB],
                            ot[:])
                return op
            for sc in range(c.S_BLK // 128):
                for oc in range(c.D // c.OB):
                    for ic in range(c.JC):
                        ops.append(mk(sc, oc, ic))
            return ops

        # ---- inline head compute: kT jc0 (all tb) + qT(0) jc0 ----
        for tb in range(c.TBn):
            for op in kT_ops(tb, 0):
                op()
        qT_tiles = {0: qpool.tile([128, c.JC * c.S_BLK], DT, tag="qT",
                                  name="qT0")}
        q0 = emit_qT_mms(0, xq_tiles[0], qT_tiles[0])
        for op in q0[0:8]:
            op()

        # ---- units + filler lists ----
        units = [(sb, hp) for sb in range(c.NSB) for hp in range(c.JC)]
        fillers = [[] for _ in units]
        # prologue fill: v rides the first two units as fillers — safe only
        # because those units' PV pops are deep-held (PVLAG0) so the PV
        # matmuls stay far behind the v' copies; kT jc1/jc2/jc3 + qT(0)
        # jc1/jc2/jc3 land one unit ahead of their consumers.
        fillers[0] += v_ops(0) + v_ops(1) + kT_ops(0, 1) + kT_ops(1, 1) \
            + kT_ops(2, 1) + kT_ops(3, 1) + q0[8:16]
        fillers[1] += v_ops(2) + v_ops(3) + kT_ops(0, 2) + kT_ops(1, 2) \
            + kT_ops(2, 2) + kT_ops(3, 2) + q0[16:24]
        fillers[2] += kT_ops(0, 3) + kT_ops(1, 3) + kT_ops(2, 3) \
            + kT_ops(3, 3) + q0[24:32]

        cat_tiles = {}

        # ---- lagged-PV queue machinery ----
        pvq = deque()        # (key, op, islast)
        stage_runs = {}      # key -> closure(cur_idx)
        normq = deque()      # deferred normalize closures
        PVLAG = 6            # max pending pv ops before forced pops (steady)
        # Deep hold for sb=0 units: their PV matmuls chase the v' filler
        # writes; keep the pop point > the PE's 64-deep reorder window behind
        # the v copies (PV LDWEIGHTS hoisting past in-flight work otherwise
        # reads stale v_sb — observed as sb-0 corruption on HW).
        PVLAG0 = 24

        def emit_stage(sb, hp, catT, pv_state, cur_idx):
            """Copy PV psums to SBUF (frees the pv banks), then defer the
            reciprocal/normalize chain so it enters the DVE queue behind the
            next few filler copies instead of head-of-line blocking them."""
            stA = stpool.tile([c.VW, c.S_BLK], f32, tag="stA",
                              name=f"stA{sb}_{hp}")
            stB = stpool.tile([c.VW, c.S_BLK], f32, tag="stB",
                              name=f"stB{sb}_{hp}")
            nc.vector.tensor_copy(stA[:], pv_state["pvA"][0:c.VW, :])
            nc.vector.tensor_copy(stB[:], pv_state["pvB"][0:c.VW, :])

            def normalize(cur_idx2):
                rtiA = rpool.tile([1, c.S_BLK], f32, tag="rtiA",
                                  name=f"rtiA{sb}_{hp}")
                rtiB = rpool.tile([1, c.S_BLK], f32, tag="rtiB",
                                  name=f"rtiB{sb}_{hp}")
                # cross-partition (row 64 -> row 0) — verified OK on HW for
                # InstReciprocal specifically.
                nc.vector.reciprocal(rtiA[:], stA[c.DK:c.DK + 1, :])
                nc.vector.reciprocal(rtiB[:], stB[c.DK:c.DK + 1, :])
                rbA = rpool.tile([c.DK, c.S_BLK], f32, tag="rbA",
                                 name=f"rbA{sb}_{hp}")
                rbB = rpool.tile([c.DK, c.S_BLK], f32, tag="rbB",
                                 name=f"rbB{sb}_{hp}")
                nc.gpsimd.partition_broadcast(rbA[:], rtiA[:])
                nc.gpsimd.partition_broadcast(rbB[:], rtiB[:])
                nc.vector.tensor_mul(
                    catT[0:c.DK, hp * c.S_BLK:(hp + 1) * c.S_BLK],
                    stA[0:c.DK, :], rbA[:])
                nc.vector.tensor_mul(
                    catT[64:64 + c.DK, hp * c.S_BLK:(hp + 1) * c.S_BLK],
                    stB[0:c.DK, :], rbB[:])
                if hp == c.JC - 1:
                    oops = emit_outproj_mms(sb, catT)
                    splits = [(0, 12), (12, 22), (22, 32)]
                    for j, (lo, hi) in enumerate(splits):
                        tgt = cur_idx2 + 1 + j
                        if tgt < len(units):
                            fillers[min(tgt, len(units) - 1)] += oops[lo:hi]
                        else:
                            for op in oops[lo:hi]:
                                op()
            normq.append(normalize)

        def make_pv_ops(key, sb, hp, t, es_tile, pv_state):
            W = c.HL * c.VW

            def opA():
                if t == 0:
                    pv_state["pvA"] = pvpool.tile([128, c.S_BLK], f32,
                                                  tag="pv", name=f"pvA{sb}_{hp}")
                nc.tensor.matmul(
                    pv_state["pvA"][0:c.VW, :],
                    v_sb[:, t * W + (2 * hp) * c.VW:
                         t * W + (2 * hp + 1) * c.VW],
                    es_tile[:, 0:c.S_BLK],
                    start=(t == 0), stop=(t == c.TCn - 1))

            def opB():
                if t == 0:
                    pv_state["pvB"] = pvpool.tile([128, c.S_BLK], f32,
                                                  tag="pv", name=f"pvB{sb}_{hp}")
                nc.tensor.matmul(
                    pv_state["pvB"][0:c.VW, :],
                    v_sb[:, t * W + (2 * hp + 1) * c.VW:
                         t * W + (2 * hp + 2) * c.VW],
                    es_tile[:, c.S_BLK:2 * c.S_BLK],
                    start=(t == 0), stop=(t == c.TCn - 1))
            need_v = t + 1 if sb == 0 else 0
            return [(key, opA, False, need_v),
                    (key, opB, t == c.TCn - 1, need_v)]

        def pop_pv(n, cur_idx):
            for _ in range(n):
                if not pvq:
                    return
                if pvq[0][3] > v_done[0]:
                    return  # its v' chunk copy not yet emitted
                key, op, islast, _ = pvq.popleft()
                op()
                if islast:
                    stage_runs.pop(key)(cur_idx)

        def pop_norm(cur_idx):
            while normq:
                normq.popleft()(cur_idx)

        # ---- main pipeline over units ----
        for idx, (sb, hp) in enumerate(units):
            if hp == 0:
                cat_tiles[sb] = cpool.tile([128, c.JC * c.S_BLK], DT,
                                           tag="cat", name=f"catT{sb}")
            if sb == 0 and hp == 2 and c.NSB > 1:
                # qT(1) fillers late in sb 0 (xq1 was head-loaded); xq2 load
                # goes to a fresh ring slot (xqpool bufs=3)
                if c.NSB > 2:
                    xq_tiles[2] = load_x(xqpool, xqT, 2, c.S_BLK, "xq2")
                qT_tiles[1] = qpool.tile([128, c.JC * c.S_BLK], DT,
                                         tag="qT", name="qT1")
                q1 = emit_qT_mms(1, xq_tiles[1], qT_tiles[1])
                fillers[idx] += q1[:16]
                fillers[min(idx + 1, len(units) - 1)] += q1[16:]
            if sb >= 1 and hp == 0 and sb + 1 < c.NSB:
                # steady state: xq(sb+2) prefetch + qT(sb+1) fillers spread
                # over all four units of this sb
                if sb + 2 < c.NSB and sb + 2 not in xq_tiles:
                    def pfq(sb=sb):
                        xq_tiles[sb + 2] = load_x(xqpool, xqT, sb + 2,
                                                  c.S_BLK, f"xq{sb + 2}")
                    fillers[idx].append(pfq)
                qT_tiles[sb + 1] = qpool.tile([128, c.JC * c.S_BLK], DT,
                                              tag="qT", name=f"qT{sb + 1}")
                qops = emit_qT_mms(sb + 1, xq_tiles[sb + 1], qT_tiles[sb + 1])
                for j in range(4):
                    fillers[min(idx + j, len(units) - 1)] += qops[j * 8:(j + 1) * 8]
            catT = cat_tiles[sb]
            qT = qT_tiles[sb]
            key = (sb, hp)
            pv_state = {}
            stage_runs[key] = (
                lambda cur_idx, sb=sb, hp=hp, catT=catT, pv_state=pv_state:
                emit_stage(sb, hp, catT, pv_state, cur_idx))
            flist = fillers[idx]
            fpos = 0
            for th in range(c.TCn // 2):
                pop_norm(idx)
                for u in range(2):
                    t = 2 * th + u
                    # deep hold while sb-0's v' fillers are in flight, then
                    # taper back to the steady lag (avoids a pop burst)
                    gch = idx * c.TCn + t
                    if sb == 0 and hp <= 1:
                        lag = PVLAG0
                    else:
                        lag = max(PVLAG, PVLAG0 - max(0, gch - 2 * c.TCn))
                    kcol = slice(hp * c.S + t * 128, hp * c.S + (t + 1) * 128)
                    qcol = slice(hp * c.S_BLK, (hp + 1) * c.S_BLK)
                    # pops BEFORE the es alloc: the exp-slot ring may reuse a
                    # slot whose readers are exactly these pv pops. Backstop:
                    # if pops are v-gated and the queue nears the es-ring
                    # capacity, pull fillers forward to advance the v copies.
                    while len(pvq) >= 26 and fpos < len(flist):
                        flist[fpos]()
                        fpos += 1
                    pop_pv(len(pvq) - lag, idx)
                    # One 2-bank psum tile [A(512) | B(512)] per t-chunk:
                    # K=64 row-tiled pair (head A rows 0-63 tile (0,0), head B
                    # rows 64-127 tile (64,0)) in different banks; ONE exp
                    # covers both heads so the pair stays adjacent/concurrent.
                    ps2 = pspool.tile([128, 2 * c.S_BLK], f32, tag="ps",
                                      name=f"ps2_{sb}_{hp}_{t}")
                    nc.tensor.matmul(
                        ps2[:, 0:c.S_BLK],
                        kT_sb[0:64, kcol], qT[0:64, qcol],
                        start=True, stop=True)
                    nc.tensor.matmul(
                        ps2[:, c.S_BLK:2 * c.S_BLK],
                        kT_sb[64:128, kcol], qT[64:128, qcol],
                        start=True, stop=True)
                    es_t = epool.tile([128, 2 * c.S_BLK], EXPDT, tag="exp",
                                      name=f"es{sb}_{hp}_{t}")
                    nc.scalar.activation(
                        es_t[:], ps2[:], mybir.ActivationFunctionType.Exp,
                        scale=SCALE)
                    pvq.extend(make_pv_ops(key, sb, hp, t, es_t, pv_state))
                want = (len(flist) * (th + 1)) // (c.TCn // 2)
                while fpos < want:
                    flist[fpos]()
                    fpos += 1
            # any fillers appended after pacing window closed
            while fpos < len(flist):
                flist[fpos]()
                fpos += 1
        # ---- drain ----
        pop_pv(len(pvq), len(units) - 1)
        pop_norm(len(units) - 1)

    nc.compile()
    return nc


def shard_inputs(inputs: dict, cfg: Cfg, DT=mybir.dt.bfloat16):
    """Full inputs -> list of 8 per-core in_maps (numpy)."""
    npdt = DT_NP[DT]
    q, k, v = inputs["queries"], inputs["keys"], inputs["values"]
    Wq, Wk, Wv = inputs["Wq"], inputs["Wk"], inputs["Wv"]
    Wout = inputs["Wout"]
    B = q.shape[0]
    maps = []
    WoutT = np.ascontiguousarray(Wout.T)  # [i, o]
    for core in range(2 * B):
        b, half = divmod(core, 2)
        hs = slice(half * cfg.HL, (half + 1) * cfg.HL)
        i0 = half * cfg.JW
        maps.append({
            "xqT": np.ascontiguousarray(q[b].T).astype(npdt),
            "xkT": np.ascontiguousarray(k[b].T).astype(npdt),
            "xvT": np.ascontiguousarray(v[b].T).astype(npdt),
            "wq": np.ascontiguousarray(
                Wq[hs].transpose(1, 0, 2).reshape(cfg.D, cfg.JW)).astype(npdt),
            "wk": np.ascontiguousarray(
                Wk[hs].transpose(1, 0, 2).reshape(cfg.D, cfg.JW)).astype(npdt),
            "wv": np.ascontiguousarray(
                Wv[hs].transpose(1, 0, 2).reshape(cfg.D, cfg.JW)).astype(npdt),
            "woutT": np.ascontiguousarray(WoutT[i0:i0 + cfg.JW]).astype(npdt),
        })
    return maps


def gather_outputs(results, inputs):
    bout = inputs["bout"]
    B = inputs["queries"].shape[0]
    outs = []
    for b in range(B):
        outs.append(results[2 * b]["out"] + results[2 * b + 1]["out"] + bout)
    return np.stack(outs).astype(np.float32)


def percore_reference(in_map: dict, cfg: Cfg):
    """Numpy reference of what one core should produce (fp32 math)."""
    c = cfg
    xq = in_map["xqT"].astype(np.float32).T   # [S, D]
    xk = in_map["xkT"].astype(np.float32).T
    xv = in_map["xvT"].astype(np.float32).T
    wq = in_map["wq"].astype(np.float32)      # [D, JW]
    wk = in_map["wk"].astype(np.float32)
    wv = in_map["wv"].astype(np.float32)
    wo = in_map["woutT"].astype(np.float32)   # [JW, D]
    q = xq @ wq                               # [S, JW]
    k = xk @ wk
    v = xv @ wv
    cat = np.zeros((c.S, c.JW), dtype=np.float32)
    for h in range(c.HL):
        sl = slice(h * c.DK, (h + 1) * c.DK)
        s = (q[:, sl] @ k[:, sl].T) / np.sqrt(c.DK)
        e = np.exp(s)
        p = e / e.sum(axis=1, keepdims=True)
        cat[:, sl] = p @ v[:, sl]
    return cat @ wo

# ----------------------------------------------------------------------------
# Self-contained entry point: kernel(**inputs) -> full [B, S, D] output.
# ----------------------------------------------------------------------------
_NC_CACHE = {}


def _get_nc():
    key = "attn"
    if key not in _NC_CACHE:
        _NC_CACHE[key] = build_nc(Cfg(), mybir.dt.bfloat16, num_devices=8)
    return _NC_CACHE[key]


def kernel(**inputs):
    """Full (unsharded) inputs -> full [4, 2048, 1024] float32 output.

    Shards across the 8 NeuronCores as (batch x head-half), runs the Bass
    kernel SPMD, and gathers: out[b] = partial(core 2b) + partial(core 2b+1)
    + bias (row-sharded fc_out -> partial-sum reduction at gather time).
    """
    from concourse.bass_utils import run_bass_kernel_spmd

    inputs = {k: np.asarray(v) for k, v in inputs.items()}
    cfg = Cfg()
    nc = _get_nc()
    maps = shard_inputs(inputs, cfg, mybir.dt.bfloat16)
    res = run_bass_kernel_spmd(nc, maps, core_ids=list(range(8)), trace=False)
    return gather_outputs(res.results, inputs)



# revision 34
# speedup vs baseline: 1.1698x; 1.0992x over previous
"""Multi-head attention Bass/Tile kernel for TRN2, sharded 8 ways.

Sharding: core c handles batch b = c//2 and heads half = c%2 (8 of 16 heads).
Each core computes, for its batch and its 8 heads:
  q/k/v projections -> scoresT = K @ Q^T (per head, [t, s] layout) -> exp ->
  PV matmul with a ones-column appended to V (gives row sums for free) ->
  normalize -> partial output projection against its 512 rows of Wout^T.
Host sums the two partials per batch and adds the bias.

Layout choices (all chosen so NO transposes are needed anywhere):
  xT     [D, S]  : host-pretransposed activations (d on partitions)
  wq/wk  [D, H*dk] : lhsT layout for qT/kT = W^T @ xT
  wv     [D, H*dk] : rhs layout for v = xT^T @ wv  ([t, vdim], natural)
  kT     [H*dk, S]: j on partitions -> head-pair p lives in 128-row chunk p
  qTz    zero-padded per head: scores contract K=128 at base partition 0,
         sharing the kT stationary operand between the pair's two matmuls
  scoresT[t, s]   : lhsT=kT [j,t], rhs=qTz [j,s]; softmax sum over t is
                    folded into the PV matmul via the ones column of v'.
  out    [s, o]   : lhsT=concatT [i,s], rhs=woutT [i,o]

The whole kernel is one software pipeline over units (sb, hp): the PV
matmuls of unit k-1 are interleaved t-chunk-wise into the scores loop of
unit k so the PE never queues behind an exp it is waiting on, and the ACT
engine (the bottleneck: 33.5M exps/core) is fed continuously. The v'
projection fills the PV slot of the very first unit.

HW pitfalls baked in (learned on-device):
  - no partition-shifting DVE copies (sim allows them, HW corrupts);
    the only cross-partition moves are InstReciprocal psum[64:65]->sbuf[0:1]
    (verified on HW) and gpsimd partition_broadcast
  - reciprocal_approx_fast (custom DVE op) produces garbage on HW
  - matmul free dim capped at 512; 2-bank psum tiles need bank-aligned halves
"""

from contextlib import ExitStack
from dataclasses import dataclass

import numpy as np
import ml_dtypes

import concourse.bass as bass  # noqa: F401
import concourse.tile as tile
from concourse import bacc, mybir


@dataclass
class Cfg:
    D: int = 1024      # model dim
    S: int = 2048      # sequence length (queries == keys)
    HL: int = 8        # heads per core
    DK: int = 64       # head dim
    S_BLK: int = 512   # query block (matmul free dim)
    T_BLK: int = 512   # t block in projection phase

    @property
    def DC(self):
        return self.D // 128

    @property
    def NSB(self):
        return self.S // self.S_BLK

    @property
    def TBn(self):
        return self.S // self.T_BLK

    @property
    def TCn(self):
        return self.S // 128

    @property
    def JW(self):
        return self.HL * self.DK

    @property
    def JC(self):
        return self.JW // 128

    @property
    def VW(self):
        return self.DK + 1

    @property
    def OB(self):
        return min(512, self.D)


DT_NP = {
    mybir.dt.bfloat16: ml_dtypes.bfloat16,
    mybir.dt.float32: np.float32,
    mybir.dt.float32r: np.float32,
}


def build_nc(cfg: Cfg, DT=mybir.dt.bfloat16, num_devices: int = 8):
    c = cfg
    f32 = mybir.dt.float32
    EXPDT = DT if DT == mybir.dt.bfloat16 else f32
    SCALE = 1.0 / float(np.sqrt(c.DK))
    nc = bacc.Bacc("TRN2", target_bir_lowering=False, debug=False,
                   num_devices=num_devices)

    xqT = nc.dram_tensor("xqT", [c.D, c.S], DT, kind="ExternalInput").ap()
    xkT = nc.dram_tensor("xkT", [c.D, c.S], DT, kind="ExternalInput").ap()
    xvT = nc.dram_tensor("xvT", [c.D, c.S], DT, kind="ExternalInput").ap()
    wq_d = nc.dram_tensor("wq", [c.D, c.JW], DT, kind="ExternalInput").ap()
    wk_d = nc.dram_tensor("wk", [c.D, c.JW], DT, kind="ExternalInput").ap()
    wv_d = nc.dram_tensor("wv", [c.D, c.JW], DT, kind="ExternalInput").ap()
    wo_d = nc.dram_tensor("woutT", [c.JW, c.D], DT, kind="ExternalInput").ap()
    out_d = nc.dram_tensor("out", [c.S, c.D], f32, kind="ExternalOutput").ap()

    from collections import deque

    with tile.TileContext(nc) as tc, ExitStack() as es:
        wpool = es.enter_context(tc.tile_pool(name="weights", bufs=1))
        kvpool = es.enter_context(tc.tile_pool(name="kv", bufs=1))
        xkpool = es.enter_context(tc.tile_pool(name="xk", bufs=4))
        xqpool = es.enter_context(tc.tile_pool(name="xq", bufs=2))
        xvpool = es.enter_context(tc.tile_pool(name="xv", bufs=2))
        qpool = es.enter_context(tc.tile_pool(name="q", bufs=2))
        epool = es.enter_context(tc.tile_pool(name="exp", bufs=18))
        cpool = es.enter_context(tc.tile_pool(name="cat", bufs=2))
        opool = es.enter_context(tc.tile_pool(name="o", bufs=2))
        rpool = es.enter_context(tc.tile_pool(name="r", bufs=1))
        stpool = es.enter_context(tc.tile_pool(name="st", bufs=2))
        pspool = es.enter_context(tc.tile_pool(name="ps", bufs=2, space="PSUM"))
        pvpool = es.enter_context(tc.tile_pool(name="pv", bufs=2, space="PSUM"))
        fppool = es.enter_context(tc.tile_pool(name="fp", bufs=2, space="PSUM"))

        def load_w_dmaj(dram, width, tag):
            t = wpool.tile([128, c.DC * width], DT, tag=tag, name=tag)
            for d in range(c.DC):
                eng = nc.sync if d % 2 == 0 else nc.gpsimd
                eng.dma_start(t[:, d * width:(d + 1) * width],
                              dram[d * 128:(d + 1) * 128, :])
            return t

        def load_x(pool, dram, blk, width, name):
            t = pool.tile([128, c.DC * width], DT, tag="x", name=name)
            for d in range(c.DC):
                eng = nc.sync if d % 2 == 0 else nc.gpsimd
                eng.dma_start(
                    t[:, d * width:(d + 1) * width],
                    dram[d * 128:(d + 1) * 128, blk * width:(blk + 1) * width])
            return t

        NT = c.T_BLK

        # ---- head DMAs: wk + xk lead (kT jc0 inline), then wq/xq0 for
        # qT(0) jc0, then wv/xv + wo for the v/outproj fillers.
        wk_sb = load_w_dmaj(wk_d, c.JW, "wk")
        kT_sb = kvpool.tile([128, c.JC * c.S], DT, tag="kT", name="kT")
        xk_tiles = [load_x(xkpool, xkT, tb, NT, f"xk{tb}")
                    for tb in range(c.TBn)]
        wq_sb = load_w_dmaj(wq_d, c.JW, "wq")
        xq_tiles = {0: load_x(xqpool, xqT, 0, c.S_BLK, "xq0")}
        wv_sb = load_w_dmaj(wv_d, c.JW, "wv")
        v_sb = kvpool.tile([128, c.TCn * c.HL * c.VW], DT, tag="v", name="v")
        nc.gpsimd.memset(v_sb[:], 1.0)  # ones columns preset
        xv_tiles = {0: load_x(xvpool, xvT, 0, NT, "xv0"),
                    1: load_x(xvpool, xvT, 1, NT, "xv1")}
        xq_tiles[1] = load_x(xqpool, xqT, 1, c.S_BLK, "xq1")
        wo_sb = wpool.tile([128, c.JC * c.D], DT, tag="wo", name="wo")
        for ic in range(c.JC):
            nc.sync.dma_start(wo_sb[:, ic * c.D:(ic + 1) * c.D],
                              wo_d[ic * 128:(ic + 1) * 128, :])

        def kT_ops(tb, jc):
            """8 MM closures computing kT chunk jc for t-block tb."""
            box = {}

            def mk(d):
                def op():
                    if d == 0:
                        box["ps"] = pspool.tile([128, NT], f32, tag="ps",
                                                name=f"psk{tb}_{jc}")
                    nc.tensor.matmul(
                        box["ps"][:],
                        wk_sb[:, d * c.JW + jc * 128: d * c.JW + (jc + 1) * 128],
                        xk_tiles[tb][:, d * NT:(d + 1) * NT],
                        start=(d == 0), stop=(d == c.DC - 1))
                    if d == c.DC - 1:
                        nc.vector.tensor_copy(
                            kT_sb[:, jc * c.S + tb * NT: jc * c.S + (tb + 1) * NT],
                            box["ps"][:])
                return op
            return [mk(d) for d in range(c.DC)]

        # count of v' chunk-groups whose SBUF copy has been EMITTED — PV pops
        # for sb-0 units must not overtake this (emission order defines the
        # read/write ordering the tracker enforces; a PV matmul emitted before
        # its v chunk's copy reads the memset ones instead).
        v_done = [0]

        def v_ops(tb):
            """v' projection closures for t-block tb (+ trailing xv prefetch —
            after the consuming MMs so the xv ring reuse sees its readers)."""
            ops = []
            for tt in range(NT // 128):
                g = tb * (NT // 128) + tt
                box = {}

                def mk(d, g=g, tt=tt, tb=tb, box=box):
                    def op():
                        if d == 0:
                            box["ps"] = pspool.tile([128, c.JW], f32, tag="ps",
                                                    name=f"psv{g}")
                        nc.tensor.matmul(
                            box["ps"][:],
                            xv_tiles[tb][:, d * NT + tt * 128:
                                         d * NT + (tt + 1) * 128],
                            wv_sb[:, d * c.JW:(d + 1) * c.JW],
                            start=(d == 0), stop=(d == c.DC - 1))
                        if d == c.DC - 1:
                            dst = v_sb[:, g * c.HL * c.VW:(g + 1) * c.HL * c.VW]
                            dst3 = dst.rearrange("p (h w) -> p h w",
                                                 w=c.VW)[:, :, 0:c.DK]
                            src3 = box["ps"][:].rearrange("p (h w) -> p h w",
                                                          w=c.DK)
                            nc.vector.tensor_copy(dst3, src3)
                            v_done[0] = g + 1
                    return op
                ops += [mk(d) for d in range(c.DC)]
            if tb + 2 < c.TBn:
                def pf(tb=tb):
                    xv_tiles[tb + 2] = load_x(xvpool, xvT, tb + 2, NT,
                                              f"xv{tb + 2}")
                ops.append(pf)
            return ops

        def emit_qT_mms(sb, xq, qT):
            """32 MM closures (jc-major); last per jc copies psum -> qT chunk
            jc (head A rows 0:64, head B rows 64:128 — natural layout)."""
            ops = []
            psq_box = {}

            def mk(jc, d):
                def op():
                    if d == 0:
                        psq_box[jc] = fppool.tile([128, c.S_BLK], f32, tag="fp",
                                                  name=f"psq{sb}_{jc}")
                    nc.tensor.matmul(
                        psq_box[jc][:],
                        wq_sb[:, d * c.JW + jc * 128: d * c.JW + (jc + 1) * 128],
                        xq[:, d * c.S_BLK:(d + 1) * c.S_BLK],
                        start=(d == 0), stop=(d == c.DC - 1))
                    if d == c.DC - 1:
                        nc.vector.tensor_copy(
                            qT[:, jc * c.S_BLK:(jc + 1) * c.S_BLK],
                            psq_box[jc][:])
                return op
            for jc in range(c.JC):
                for d in range(c.DC):
                    ops.append(mk(jc, d))
            return ops

        def emit_outproj_mms(sb, catT):
            """Closures: per (sc, oc): 4 ic-MMs into a 1-bank psum, then
            copy + DMA out."""
            ops = []
            po_box = {}

            def mk(sc, oc, ic):
                def op():
                    if ic == 0:
                        po_box[(sc, oc)] = fppool.tile(
                            [128, c.OB], f32, tag="fp", name=f"po{sb}_{sc}_{oc}")
                    po = po_box[(sc, oc)]
                    nc.tensor.matmul(
                        po[:],
                        catT[:, ic * c.S_BLK + sc * 128:
                             ic * c.S_BLK + (sc + 1) * 128],
                        wo_sb[:, ic * c.D + oc * c.OB:
                              ic * c.D + (oc + 1) * c.OB],
                        start=(ic == 0), stop=(ic == c.JC - 1))
                    if ic == c.JC - 1:
                        ot = opool.tile([128, c.OB], f32, tag="ot",
                                        name=f"ot{sb}_{sc}_{oc}")
                        nc.vector.tensor_copy(ot[:], po[:])
                        nc.sync.dma_start(
                            out_d[sb * c.S_BLK + sc * 128:
                                  sb * c.S_BLK + (sc + 1) * 128,
                                  oc * c.OB:(oc + 1) * c.OB],
                            ot[:])
                return op
            for sc in range(c.S_BLK // 128):
                for oc in range(c.D // c.OB):
                    for ic in range(c.JC):
                        ops.append(mk(sc, oc, ic))
            return ops

        # ---- inline head compute: kT jc0 (all tb) + qT(0) jc0 ----
        for tb in range(c.TBn):
            for op in kT_ops(tb, 0):
                op()
        qT_tiles = {0: qpool.tile([128, c.JC * c.S_BLK], DT, tag="qT",
                                  name="qT0")}
        q0 = emit_qT_mms(0, xq_tiles[0], qT_tiles[0])
        for op in q0[0:8]:
            op()

        # ---- units + filler lists ----
        units = [(sb, hp) for sb in range(c.NSB) for hp in range(c.JC)]
        fillers = [[] for _ in units]
        # prologue fill: v rides the first two units as fillers — safe only
        # because those units' PV pops are deep-held (PVLAG0) so the PV
        # matmuls stay far behind the v' copies; kT jc1/jc2/jc3 + qT(0)
        # jc1/jc2/jc3 land one unit ahead of their consumers.
        fillers[0] += v_ops(0) + v_ops(1) + kT_ops(0, 1) + kT_ops(1, 1) \
            + kT_ops(2, 1) + kT_ops(3, 1) + q0[8:16]
        fillers[1] += v_ops(2) + v_ops(3) + kT_ops(0, 2) + kT_ops(1, 2) \
            + kT_ops(2, 2) + kT_ops(3, 2) + q0[16:24]
        fillers[2] += kT_ops(0, 3) + kT_ops(1, 3) + kT_ops(2, 3) \
            + kT_ops(3, 3) + q0[24:32]

        cat_tiles = {}

        # ---- lagged-PV queue machinery ----
        pvq = deque()        # (key, op, islast, need_v)
        stage_runs = {}      # key -> closure(cur_idx)
        normq = deque()      # deferred normalize closures
        pending_tail = []    # ops deferred past the last unit (drain)
        PVLAG = 32           # one full unit behind (PV(k) pops during k+1)
        # Deep hold for sb=0 units: their PV matmuls chase the v' filler
        # writes; keep the pop point > the PE's 64-deep reorder window behind
        # the v copies (PV LDWEIGHTS hoisting past in-flight work otherwise
        # reads stale v_sb — observed as sb-0 corruption on HW).
        PVLAG0 = 32

        def emit_stage(sb, hp, catT, pv_state, cur_idx):
            """Copy PV psums to SBUF (frees the pv banks), then defer the
            reciprocal/normalize chain so it enters the DVE queue behind the
            next few filler copies instead of head-of-line blocking them."""
            stA = stpool.tile([c.VW, c.S_BLK], f32, tag="stA",
                              name=f"stA{sb}_{hp}")
            stB = stpool.tile([c.VW, c.S_BLK], f32, tag="stB",
                              name=f"stB{sb}_{hp}")
            nc.vector.tensor_copy(stA[:], pv_state["pvA"][0:c.VW, :])
            nc.vector.tensor_copy(stB[:], pv_state["pvB"][0:c.VW, :])

            def normalize(cur_idx2):
                rtiA = rpool.tile([1, c.S_BLK], f32, tag="rtiA",
                                  name=f"rtiA{sb}_{hp}")
                rtiB = rpool.tile([1, c.S_BLK], f32, tag="rtiB",
                                  name=f"rtiB{sb}_{hp}")
                # cross-partition (row 64 -> row 0) — verified OK on HW for
                # InstReciprocal specifically.
                nc.vector.reciprocal(rtiA[:], stA[c.DK:c.DK + 1, :])
                nc.vector.reciprocal(rtiB[:], stB[c.DK:c.DK + 1, :])
                rbA = rpool.tile([c.DK, c.S_BLK], f32, tag="rbA",
                                 name=f"rbA{sb}_{hp}")
                rbB = rpool.tile([c.DK, c.S_BLK], f32, tag="rbB",
                                 name=f"rbB{sb}_{hp}")
                nc.gpsimd.partition_broadcast(rbA[:], rtiA[:])
                nc.gpsimd.partition_broadcast(rbB[:], rtiB[:])
                nc.vector.tensor_mul(
                    catT[0:c.DK, hp * c.S_BLK:(hp + 1) * c.S_BLK],
                    stA[0:c.DK, :], rbA[:])
                nc.vector.tensor_mul(
                    catT[64:64 + c.DK, hp * c.S_BLK:(hp + 1) * c.S_BLK],
                    stB[0:c.DK, :], rbB[:])
                if hp == c.JC - 1:
                    oops = emit_outproj_mms(sb, catT)
                    splits = [(0, 12), (12, 22), (22, 32)]
                    for j, (lo, hi) in enumerate(splits):
                        tgt = cur_idx2 + 1 + j
                        if tgt < len(units):
                            fillers[tgt] += oops[lo:hi]
                        else:
                            # keep ic-order: never run a later split inline
                            # while an earlier one sits in a filler list
                            pending_tail.extend(oops[lo:hi])
            normq.append(normalize)

        def make_pv_ops(key, sb, hp, t, es_tile, pv_state):
            W = c.HL * c.VW

            def opA():
                if t == 0:
                    pv_state["pvA"] = pvpool.tile([128, c.S_BLK], f32,
                                                  tag="pv", name=f"pvA{sb}_{hp}")
                nc.tensor.matmul(
                    pv_state["pvA"][0:c.VW, :],
                    v_sb[:, t * W + (2 * hp) * c.VW:
                         t * W + (2 * hp + 1) * c.VW],
                    es_tile[:, 0:c.S_BLK],
                    start=(t == 0), stop=(t == c.TCn - 1))

            def opB():
                if t == 0:
                    pv_state["pvB"] = pvpool.tile([128, c.S_BLK], f32,
                                                  tag="pv", name=f"pvB{sb}_{hp}")
                nc.tensor.matmul(
                    pv_state["pvB"][0:c.VW, :],
                    v_sb[:, t * W + (2 * hp + 1) * c.VW:
                         t * W + (2 * hp + 2) * c.VW],
                    es_tile[:, c.S_BLK:2 * c.S_BLK],
                    start=(t == 0), stop=(t == c.TCn - 1))
            need_v = t + 1 if sb == 0 else 0
            return [(key, opA, False, need_v),
                    (key, opB, t == c.TCn - 1, need_v)]

        def pop_pv(n, cur_idx):
            for _ in range(n):
                if not pvq:
                    return
                if pvq[0][3] > v_done[0]:
                    return  # its v' chunk copy not yet emitted
                key, op, islast, _ = pvq.popleft()
                op()
                if islast:
                    stage_runs.pop(key)(cur_idx)

        def pop_norm(cur_idx):
            while normq:
                normq.popleft()(cur_idx)

        # ---- main pipeline over units ----
        for idx, (sb, hp) in enumerate(units):
            if hp == 0:
                cat_tiles[sb] = cpool.tile([128, c.JC * c.S_BLK], DT,
                                           tag="cat", name=f"catT{sb}")
            if sb == 0 and hp == 3 and c.NSB > 2:
                # xq2 load here: xq0's ring slot is free (all qT(0) MMs were
                # emitted by the end of unit (0,2))
                xq_tiles[2] = load_x(xqpool, xqT, 2, c.S_BLK, "xq2")
            if sb == 0 and hp == 2 and c.NSB > 1:
                qT_tiles[1] = qpool.tile([128, c.JC * c.S_BLK], DT,
                                         tag="qT", name="qT1")
                q1 = emit_qT_mms(1, xq_tiles[1], qT_tiles[1])
                fillers[idx] += q1[:16]
                fillers[min(idx + 1, len(units) - 1)] += q1[16:]
            if sb >= 1 and hp == 0 and sb + 1 < c.NSB:
                # steady state: xq(sb+2) prefetch + qT(sb+1) fillers spread
                # over all four units of this sb
                if sb + 2 < c.NSB and sb + 2 not in xq_tiles:
                    def pfq(sb=sb):
                        xq_tiles[sb + 2] = load_x(xqpool, xqT, sb + 2,
                                                  c.S_BLK, f"xq{sb + 2}")
                    fillers[idx].append(pfq)
                qT_tiles[sb + 1] = qpool.tile([128, c.JC * c.S_BLK], DT,
                                              tag="qT", name=f"qT{sb + 1}")
                qops = emit_qT_mms(sb + 1, xq_tiles[sb + 1], qT_tiles[sb + 1])
                for j in range(4):
                    fillers[min(idx + j, len(units) - 1)] += qops[j * 8:(j + 1) * 8]
            catT = cat_tiles[sb]
            qT = qT_tiles[sb]
            key = (sb, hp)
            pv_state = {}
            stage_runs[key] = (
                lambda cur_idx, sb=sb, hp=hp, catT=catT, pv_state=pv_state:
                emit_stage(sb, hp, catT, pv_state, cur_idx))
            flist = fillers[idx]
            fpos = 0
            for th in range(c.TCn // 2):
                pop_norm(idx)
                for u in range(2):
                    t = 2 * th + u
                    # deep hold while sb-0's v' fillers are in flight, then
                    # taper back to the steady lag (avoids a pop burst)
                    gch = idx * c.TCn + t
                    if sb == 0 and hp <= 1:
                        lag = PVLAG0
                    else:
                        lag = max(PVLAG, PVLAG0 - max(0, gch - 2 * c.TCn))
                    kcol = slice(hp * c.S + t * 128, hp * c.S + (t + 1) * 128)
                    qcol = slice(hp * c.S_BLK, (hp + 1) * c.S_BLK)
                    # pops BEFORE the es alloc: the exp-slot ring may reuse a
                    # slot whose readers are exactly these pv pops. Backstop:
                    # if pops are v-gated and the queue nears the es-ring
                    # capacity, pull fillers forward to advance the v copies.
                    while len(pvq) >= 40 and fpos < len(flist):
                        flist[fpos]()
                        fpos += 1
                    pop_pv(len(pvq) - lag, idx)
                    # One 2-bank psum tile [A(512) | B(512)] per t-chunk:
                    # K=64 row-tiled pair (head A rows 0-63 tile (0,0), head B
                    # rows 64-127 tile (64,0)) in different banks; ONE exp
                    # covers both heads so the pair stays adjacent/concurrent.
                    ps2 = pspool.tile([128, 2 * c.S_BLK], f32, tag="ps",
                                      name=f"ps2_{sb}_{hp}_{t}")
                    nc.tensor.matmul(
                        ps2[:, 0:c.S_BLK],
                        kT_sb[0:64, kcol], qT[0:64, qcol],
                        start=True, stop=True)
                    nc.tensor.matmul(
                        ps2[:, c.S_BLK:2 * c.S_BLK],
                        kT_sb[64:128, kcol], qT[64:128, qcol],
                        start=True, stop=True)
                    es_t = epool.tile([128, 2 * c.S_BLK], EXPDT, tag="exp",
                                      name=f"es{sb}_{hp}_{t}")
                    nc.scalar.activation(
                        es_t[:], ps2[:], mybir.ActivationFunctionType.Exp,
                        scale=SCALE)
                    pvq.extend(make_pv_ops(key, sb, hp, t, es_t, pv_state))
                want = (len(flist) * (th + 1)) // (c.TCn // 2)
                while fpos < want:
                    flist[fpos]()
                    fpos += 1
            # any fillers appended after pacing window closed
            while fpos < len(flist):
                flist[fpos]()
                fpos += 1
        # ---- drain ----
        pop_pv(len(pvq), len(units) - 1)
        pop_norm(len(units) - 1)
        for op in pending_tail:
            op()

    nc.compile()
    return nc


def shard_inputs(inputs: dict, cfg: Cfg, DT=mybir.dt.bfloat16):
    """Full inputs -> list of 8 per-core in_maps (numpy)."""
    npdt = DT_NP[DT]
    q, k, v = inputs["queries"], inputs["keys"], inputs["values"]
    Wq, Wk, Wv = inputs["Wq"], inputs["Wk"], inputs["Wv"]
    Wout = inputs["Wout"]
    B = q.shape[0]
    maps = []
    WoutT = np.ascontiguousarray(Wout.T)  # [i, o]
    for core in range(2 * B):
        b, half = divmod(core, 2)
        hs = slice(half * cfg.HL, (half + 1) * cfg.HL)
        i0 = half * cfg.JW
        maps.append({
            "xqT": np.ascontiguousarray(q[b].T).astype(npdt),
            "xkT": np.ascontiguousarray(k[b].T).astype(npdt),
            "xvT": np.ascontiguousarray(v[b].T).astype(npdt),
            "wq": np.ascontiguousarray(
                Wq[hs].transpose(1, 0, 2).reshape(cfg.D, cfg.JW)).astype(npdt),
            "wk": np.ascontiguousarray(
                Wk[hs].transpose(1, 0, 2).reshape(cfg.D, cfg.JW)).astype(npdt),
            "wv": np.ascontiguousarray(
                Wv[hs].transpose(1, 0, 2).reshape(cfg.D, cfg.JW)).astype(npdt),
            "woutT": np.ascontiguousarray(WoutT[i0:i0 + cfg.JW]).astype(npdt),
        })
    return maps


def gather_outputs(results, inputs):
    bout = inputs["bout"]
    B = inputs["queries"].shape[0]
    outs = []
    for b in range(B):
        outs.append(results[2 * b]["out"] + results[2 * b + 1]["out"] + bout)
    return np.stack(outs).astype(np.float32)


def percore_reference(in_map: dict, cfg: Cfg):
    """Numpy reference of what one core should produce (fp32 math)."""
    c = cfg
    xq = in_map["xqT"].astype(np.float32).T   # [S, D]
    xk = in_map["xkT"].astype(np.float32).T
    xv = in_map["xvT"].astype(np.float32).T
    wq = in_map["wq"].astype(np.float32)      # [D, JW]
    wk = in_map["wk"].astype(np.float32)
    wv = in_map["wv"].astype(np.float32)
    wo = in_map["woutT"].astype(np.float32)   # [JW, D]
    q = xq @ wq                               # [S, JW]
    k = xk @ wk
    v = xv @ wv
    cat = np.zeros((c.S, c.JW), dtype=np.float32)
    for h in range(c.HL):
        sl = slice(h * c.DK, (h + 1) * c.DK)
        s = (q[:, sl] @ k[:, sl].T) / np.sqrt(c.DK)
        e = np.exp(s)
        p = e / e.sum(axis=1, keepdims=True)
        cat[:, sl] = p @ v[:, sl]
    return cat @ wo

# ----------------------------------------------------------------------------
# Self-contained entry point: kernel(**inputs) -> full [B, S, D] output.
# ----------------------------------------------------------------------------
_NC_CACHE = {}


def _get_nc():
    key = "attn"
    if key not in _NC_CACHE:
        _NC_CACHE[key] = build_nc(Cfg(), mybir.dt.bfloat16, num_devices=8)
    return _NC_CACHE[key]


def kernel(**inputs):
    """Full (unsharded) inputs -> full [4, 2048, 1024] float32 output.

    Shards across the 8 NeuronCores as (batch x head-half), runs the Bass
    kernel SPMD, and gathers: out[b] = partial(core 2b) + partial(core 2b+1)
    + bias (row-sharded fc_out -> partial-sum reduction at gather time).
    """
    from concourse.bass_utils import run_bass_kernel_spmd

    inputs = {k: np.asarray(v) for k, v in inputs.items()}
    cfg = Cfg()
    nc = _get_nc()
    maps = shard_inputs(inputs, cfg, mybir.dt.bfloat16)
    res = run_bass_kernel_spmd(nc, maps, core_ids=list(range(8)), trace=False)
    return gather_outputs(res.results, inputs)



# revision 36
# speedup vs baseline: 1.1752x; 1.0046x over previous
"""Multi-head attention Bass/Tile kernel for TRN2, sharded 8 ways.

Sharding: core c handles batch b = c//2 and heads half = c%2 (8 of 16 heads).
Each core computes, for its batch and its 8 heads:
  q/k/v projections -> scoresT = K @ Q^T (per head, [t, s] layout) -> exp ->
  PV matmul with a ones-column appended to V (gives row sums for free) ->
  normalize -> partial output projection against its 512 rows of Wout^T.
Host sums the two partials per batch and adds the bias.

Layout choices (all chosen so NO transposes are needed anywhere):
  xT     [D, S]  : host-pretransposed activations (d on partitions)
  wq/wk  [D, H*dk] : lhsT layout for qT/kT = W^T @ xT
  wv     [D, H*dk] : rhs layout for v = xT^T @ wv  ([t, vdim], natural)
  kT     [H*dk, S]: j on partitions -> head-pair p lives in 128-row chunk p
  qTz    zero-padded per head: scores contract K=128 at base partition 0,
         sharing the kT stationary operand between the pair's two matmuls
  scoresT[t, s]   : lhsT=kT [j,t], rhs=qTz [j,s]; softmax sum over t is
                    folded into the PV matmul via the ones column of v'.
  out    [s, o]   : lhsT=concatT [i,s], rhs=woutT [i,o]

The whole kernel is one software pipeline over units (sb, hp): the PV
matmuls of unit k-1 are interleaved t-chunk-wise into the scores loop of
unit k so the PE never queues behind an exp it is waiting on, and the ACT
engine (the bottleneck: 33.5M exps/core) is fed continuously. The v'
projection fills the PV slot of the very first unit.

HW pitfalls baked in (learned on-device):
  - no partition-shifting DVE copies (sim allows them, HW corrupts);
    the only cross-partition moves are InstReciprocal psum[64:65]->sbuf[0:1]
    (verified on HW) and gpsimd partition_broadcast
  - reciprocal_approx_fast (custom DVE op) produces garbage on HW
  - matmul free dim capped at 512; 2-bank psum tiles need bank-aligned halves
"""

from contextlib import ExitStack
from dataclasses import dataclass

import numpy as np
import ml_dtypes

import concourse.bass as bass  # noqa: F401
import concourse.tile as tile
from concourse import bacc, mybir


@dataclass
class Cfg:
    D: int = 1024      # model dim
    S: int = 2048      # sequence length (queries == keys)
    HL: int = 8        # heads per core
    DK: int = 64       # head dim
    S_BLK: int = 512   # query block (matmul free dim)
    T_BLK: int = 512   # t block in projection phase

    @property
    def DC(self):
        return self.D // 128

    @property
    def NSB(self):
        return self.S // self.S_BLK

    @property
    def TBn(self):
        return self.S // self.T_BLK

    @property
    def TCn(self):
        return self.S // 128

    @property
    def JW(self):
        return self.HL * self.DK

    @property
    def JC(self):
        return self.JW // 128

    @property
    def VW(self):
        return self.DK + 1

    @property
    def OB(self):
        return min(512, self.D)


DT_NP = {
    mybir.dt.bfloat16: ml_dtypes.bfloat16,
    mybir.dt.float32: np.float32,
    mybir.dt.float32r: np.float32,
}


def build_nc(cfg: Cfg, DT=mybir.dt.bfloat16, num_devices: int = 8):
    c = cfg
    f32 = mybir.dt.float32
    EXPDT = DT if DT == mybir.dt.bfloat16 else f32
    SCALE = 1.0 / float(np.sqrt(c.DK))
    nc = bacc.Bacc("TRN2", target_bir_lowering=False, debug=False,
                   num_devices=num_devices)

    xqT = nc.dram_tensor("xqT", [c.D, c.S], DT, kind="ExternalInput").ap()
    xkT = nc.dram_tensor("xkT", [c.D, c.S], DT, kind="ExternalInput").ap()
    xvT = nc.dram_tensor("xvT", [c.D, c.S], DT, kind="ExternalInput").ap()
    wq_d = nc.dram_tensor("wq", [c.D, c.JW], DT, kind="ExternalInput").ap()
    wk_d = nc.dram_tensor("wk", [c.D, c.JW], DT, kind="ExternalInput").ap()
    wv_d = nc.dram_tensor("wv", [c.D, c.JW], DT, kind="ExternalInput").ap()
    wo_d = nc.dram_tensor("woutT", [c.JW, c.D], DT, kind="ExternalInput").ap()
    out_d = nc.dram_tensor("out", [c.S, c.D], f32, kind="ExternalOutput").ap()

    from collections import deque

    with tile.TileContext(nc) as tc, ExitStack() as es:
        wpool = es.enter_context(tc.tile_pool(name="weights", bufs=1))
        kvpool = es.enter_context(tc.tile_pool(name="kv", bufs=1))
        xkpool = es.enter_context(tc.tile_pool(name="xk", bufs=4))
        xqpool = es.enter_context(tc.tile_pool(name="xq", bufs=2))
        xvpool = es.enter_context(tc.tile_pool(name="xv", bufs=2))
        qpool = es.enter_context(tc.tile_pool(name="q", bufs=2))
        epool = es.enter_context(tc.tile_pool(name="exp", bufs=18))
        cpool = es.enter_context(tc.tile_pool(name="cat", bufs=2))
        opool = es.enter_context(tc.tile_pool(name="o", bufs=2))
        rpool = es.enter_context(tc.tile_pool(name="r", bufs=1))
        stpool = es.enter_context(tc.tile_pool(name="st", bufs=2))
        pspool = es.enter_context(tc.tile_pool(name="ps", bufs=2, space="PSUM"))
        pvpool = es.enter_context(tc.tile_pool(name="pv", bufs=2, space="PSUM"))
        fppool = es.enter_context(tc.tile_pool(name="fp", bufs=2, space="PSUM"))

        def load_w_dmaj(dram, width, tag):
            t = wpool.tile([128, c.DC * width], DT, tag=tag, name=tag)
            for d in range(c.DC):
                eng = nc.sync if d % 2 == 0 else nc.gpsimd
                eng.dma_start(t[:, d * width:(d + 1) * width],
                              dram[d * 128:(d + 1) * 128, :])
            return t

        def load_x(pool, dram, blk, width, name):
            t = pool.tile([128, c.DC * width], DT, tag="x", name=name)
            for d in range(c.DC):
                eng = nc.sync if d % 2 == 0 else nc.gpsimd
                eng.dma_start(
                    t[:, d * width:(d + 1) * width],
                    dram[d * 128:(d + 1) * 128, blk * width:(blk + 1) * width])
            return t

        NT = c.T_BLK

        # ---- head DMAs: wk + xk lead (kT jc0 inline), then wq/xq0 for
        # qT(0) jc0, then wv/xv + wo for the v/outproj fillers.
        wk_sb = load_w_dmaj(wk_d, c.JW, "wk")
        kT_sb = kvpool.tile([128, c.JC * c.S], DT, tag="kT", name="kT")
        xk_tiles = [load_x(xkpool, xkT, tb, NT, f"xk{tb}")
                    for tb in range(c.TBn)]
        wq_sb = load_w_dmaj(wq_d, c.JW, "wq")
        xq_tiles = {0: load_x(xqpool, xqT, 0, c.S_BLK, "xq0")}
        wv_sb = load_w_dmaj(wv_d, c.JW, "wv")
        v_sb = kvpool.tile([128, c.TCn * c.HL * c.VW], DT, tag="v", name="v")
        nc.gpsimd.memset(v_sb[:], 1.0)  # ones columns preset
        xv_tiles = {0: load_x(xvpool, xvT, 0, NT, "xv0"),
                    1: load_x(xvpool, xvT, 1, NT, "xv1")}
        xq_tiles[1] = load_x(xqpool, xqT, 1, c.S_BLK, "xq1")
        wo_sb = wpool.tile([128, c.JC * c.D], DT, tag="wo", name="wo")
        for ic in range(c.JC):
            nc.sync.dma_start(wo_sb[:, ic * c.D:(ic + 1) * c.D],
                              wo_d[ic * 128:(ic + 1) * 128, :])

        def kT_ops(tb, jc):
            """8 MM closures computing kT chunk jc for t-block tb."""
            box = {}

            def mk(d):
                def op():
                    if d == 0:
                        box["ps"] = pspool.tile([128, NT], f32, tag="ps",
                                                name=f"psk{tb}_{jc}")
                    nc.tensor.matmul(
                        box["ps"][:],
                        wk_sb[:, d * c.JW + jc * 128: d * c.JW + (jc + 1) * 128],
                        xk_tiles[tb][:, d * NT:(d + 1) * NT],
                        start=(d == 0), stop=(d == c.DC - 1))
                    if d == c.DC - 1:
                        nc.vector.tensor_copy(
                            kT_sb[:, jc * c.S + tb * NT: jc * c.S + (tb + 1) * NT],
                            box["ps"][:])
                return op
            return [mk(d) for d in range(c.DC)]

        # count of v' chunk-groups whose SBUF copy has been EMITTED — PV pops
        # for sb-0 units must not overtake this (emission order defines the
        # read/write ordering the tracker enforces; a PV matmul emitted before
        # its v chunk's copy reads the memset ones instead).
        v_done = [0]

        def v_ops(tb):
            """v' projection closures for t-block tb (+ trailing xv prefetch —
            after the consuming MMs so the xv ring reuse sees its readers)."""
            ops = []
            for tt in range(NT // 128):
                g = tb * (NT // 128) + tt
                box = {}

                def mk(d, g=g, tt=tt, tb=tb, box=box):
                    def op():
                        if d == 0:
                            box["ps"] = pspool.tile([128, c.JW], f32, tag="ps",
                                                    name=f"psv{g}")
                        nc.tensor.matmul(
                            box["ps"][:],
                            xv_tiles[tb][:, d * NT + tt * 128:
                                         d * NT + (tt + 1) * 128],
                            wv_sb[:, d * c.JW:(d + 1) * c.JW],
                            start=(d == 0), stop=(d == c.DC - 1))
                        if d == c.DC - 1:
                            dst = v_sb[:, g * c.HL * c.VW:(g + 1) * c.HL * c.VW]
                            dst3 = dst.rearrange("p (h w) -> p h w",
                                                 w=c.VW)[:, :, 0:c.DK]
                            src3 = box["ps"][:].rearrange("p (h w) -> p h w",
                                                          w=c.DK)
                            nc.vector.tensor_copy(dst3, src3)
                            v_done[0] = g + 1
                    return op
                ops += [mk(d) for d in range(c.DC)]
            if tb + 2 < c.TBn:
                def pf(tb=tb):
                    xv_tiles[tb + 2] = load_x(xvpool, xvT, tb + 2, NT,
                                              f"xv{tb + 2}")
                ops.append(pf)
            return ops

        def emit_qT_mms(sb, xq, qT):
            """32 MM closures (jc-major); last per jc copies psum -> qT chunk
            jc (head A rows 0:64, head B rows 64:128 — natural layout)."""
            ops = []
            psq_box = {}

            def mk(jc, d):
                def op():
                    if d == 0:
                        psq_box[jc] = fppool.tile([128, c.S_BLK], f32, tag="fp",
                                                  name=f"psq{sb}_{jc}")
                    nc.tensor.matmul(
                        psq_box[jc][:],
                        wq_sb[:, d * c.JW + jc * 128: d * c.JW + (jc + 1) * 128],
                        xq[:, d * c.S_BLK:(d + 1) * c.S_BLK],
                        start=(d == 0), stop=(d == c.DC - 1))
                    if d == c.DC - 1:
                        nc.vector.tensor_copy(
                            qT[:, jc * c.S_BLK:(jc + 1) * c.S_BLK],
                            psq_box[jc][:])
                return op
            for jc in range(c.JC):
                for d in range(c.DC):
                    ops.append(mk(jc, d))
            return ops

        def emit_outproj_mms(sb, catT):
            """Closures: per (sc, oc): 4 ic-MMs into a 1-bank psum, then
            copy + DMA out."""
            ops = []
            po_box = {}

            def mk(sc, oc, ic):
                def op():
                    if ic == 0:
                        po_box[(sc, oc)] = fppool.tile(
                            [128, c.OB], f32, tag="fp", name=f"po{sb}_{sc}_{oc}")
                    po = po_box[(sc, oc)]
                    nc.tensor.matmul(
                        po[:],
                        catT[:, ic * c.S_BLK + sc * 128:
                             ic * c.S_BLK + (sc + 1) * 128],
                        wo_sb[:, ic * c.D + oc * c.OB:
                              ic * c.D + (oc + 1) * c.OB],
                        start=(ic == 0), stop=(ic == c.JC - 1))
                    if ic == c.JC - 1:
                        ot = opool.tile([128, c.OB], f32, tag="ot",
                                        name=f"ot{sb}_{sc}_{oc}")
                        nc.vector.tensor_copy(ot[:], po[:])
                        nc.sync.dma_start(
                            out_d[sb * c.S_BLK + sc * 128:
                                  sb * c.S_BLK + (sc + 1) * 128,
                                  oc * c.OB:(oc + 1) * c.OB],
                            ot[:])
                return op
            for sc in range(c.S_BLK // 128):
                for oc in range(c.D // c.OB):
                    for ic in range(c.JC):
                        ops.append(mk(sc, oc, ic))
            return ops

        # ---- inline head compute (rides under the head DMA): kT jc0+jc1
        # (all tb) + qT(0) jc0+jc1 ----
        for tb in range(c.TBn):
            for op in kT_ops(tb, 0):
                op()
        qT_tiles = {0: qpool.tile([128, c.JC * c.S_BLK], DT, tag="qT",
                                  name="qT0")}
        q0 = emit_qT_mms(0, xq_tiles[0], qT_tiles[0])
        for op in q0[0:8]:
            op()
        for tb in range(c.TBn):
            for op in kT_ops(tb, 1):
                op()
        for op in q0[8:16]:
            op()

        # ---- units + filler lists ----
        units = [(sb, hp) for sb in range(c.NSB) for hp in range(c.JC)]
        fillers = [[] for _ in units]
        # prologue fill: v rides the first two units as fillers — safe only
        # because those units' PV pops are deep-held (PVLAG0) so the PV
        # matmuls stay far behind the v' copies; kT jc1/jc2/jc3 + qT(0)
        # jc1/jc2/jc3 land one unit ahead of their consumers.
        fillers[0] += v_ops(0) + v_ops(1) + kT_ops(0, 2) + kT_ops(1, 2) \
            + kT_ops(2, 2) + kT_ops(3, 2) + q0[16:24]
        fillers[1] += v_ops(2) + v_ops(3) + kT_ops(0, 3) + kT_ops(1, 3) \
            + kT_ops(2, 3) + kT_ops(3, 3) + q0[24:32]

        cat_tiles = {}

        # ---- lagged-PV queue machinery ----
        pvq = deque()        # (key, op, islast, need_v)
        stage_runs = {}      # key -> closure(cur_idx)
        normq = deque()      # deferred normalize closures
        pending_tail = []    # ops deferred past the last unit (drain)
        PVLAG = 32           # one full unit behind (PV(k) pops during k+1)
        # Deep hold for sb=0 units: their PV matmuls chase the v' filler
        # writes; keep the pop point > the PE's 64-deep reorder window behind
        # the v copies (PV LDWEIGHTS hoisting past in-flight work otherwise
        # reads stale v_sb — observed as sb-0 corruption on HW).
        PVLAG0 = 32

        def emit_stage(sb, hp, catT, pv_state, cur_idx):
            """Copy PV psums to SBUF (frees the pv banks), then defer the
            reciprocal/normalize chain so it enters the DVE queue behind the
            next few filler copies instead of head-of-line blocking them."""
            stA = stpool.tile([c.VW, c.S_BLK], f32, tag="stA",
                              name=f"stA{sb}_{hp}")
            stB = stpool.tile([c.VW, c.S_BLK], f32, tag="stB",
                              name=f"stB{sb}_{hp}")
            nc.vector.tensor_copy(stA[:], pv_state["pvA"][0:c.VW, :])
            nc.vector.tensor_copy(stB[:], pv_state["pvB"][0:c.VW, :])

            def normalize(cur_idx2):
                rtiA = rpool.tile([1, c.S_BLK], f32, tag="rtiA",
                                  name=f"rtiA{sb}_{hp}")
                rtiB = rpool.tile([1, c.S_BLK], f32, tag="rtiB",
                                  name=f"rtiB{sb}_{hp}")
                # cross-partition (row 64 -> row 0) — verified OK on HW for
                # InstReciprocal specifically.
                nc.vector.reciprocal(rtiA[:], stA[c.DK:c.DK + 1, :])
                nc.vector.reciprocal(rtiB[:], stB[c.DK:c.DK + 1, :])
                rbA = rpool.tile([c.DK, c.S_BLK], f32, tag="rbA",
                                 name=f"rbA{sb}_{hp}")
                rbB = rpool.tile([c.DK, c.S_BLK], f32, tag="rbB",
                                 name=f"rbB{sb}_{hp}")
                nc.gpsimd.partition_broadcast(rbA[:], rtiA[:])
                nc.gpsimd.partition_broadcast(rbB[:], rtiB[:])
                nc.vector.tensor_mul(
                    catT[0:c.DK, hp * c.S_BLK:(hp + 1) * c.S_BLK],
                    stA[0:c.DK, :], rbA[:])
                nc.vector.tensor_mul(
                    catT[64:64 + c.DK, hp * c.S_BLK:(hp + 1) * c.S_BLK],
                    stB[0:c.DK, :], rbB[:])
                if hp == c.JC - 1:
                    oops = emit_outproj_mms(sb, catT)
                    splits = [(0, 12), (12, 22), (22, 32)]
                    for j, (lo, hi) in enumerate(splits):
                        tgt = cur_idx2 + 1 + j
                        if tgt < len(units):
                            fillers[tgt] += oops[lo:hi]
                        else:
                            # keep ic-order: never run a later split inline
                            # while an earlier one sits in a filler list
                            pending_tail.extend(oops[lo:hi])
            normq.append(normalize)

        def make_pv_ops(key, sb, hp, t, es_tile, pv_state):
            W = c.HL * c.VW

            def opA():
                if t == 0:
                    pv_state["pvA"] = pvpool.tile([128, c.S_BLK], f32,
                                                  tag="pv", name=f"pvA{sb}_{hp}")
                nc.tensor.matmul(
                    pv_state["pvA"][0:c.VW, :],
                    v_sb[:, t * W + (2 * hp) * c.VW:
                         t * W + (2 * hp + 1) * c.VW],
                    es_tile[:, 0:c.S_BLK],
                    start=(t == 0), stop=(t == c.TCn - 1))

            def opB():
                if t == 0:
                    pv_state["pvB"] = pvpool.tile([128, c.S_BLK], f32,
                                                  tag="pv", name=f"pvB{sb}_{hp}")
                nc.tensor.matmul(
                    pv_state["pvB"][0:c.VW, :],
                    v_sb[:, t * W + (2 * hp + 1) * c.VW:
                         t * W + (2 * hp + 2) * c.VW],
                    es_tile[:, c.S_BLK:2 * c.S_BLK],
                    start=(t == 0), stop=(t == c.TCn - 1))
            need_v = t + 1 if sb == 0 else 0
            return [(key, opA, False, need_v),
                    (key, opB, t == c.TCn - 1, need_v)]

        def pop_pv(n, cur_idx):
            for _ in range(n):
                if not pvq:
                    return
                if pvq[0][3] > v_done[0]:
                    return  # its v' chunk copy not yet emitted
                key, op, islast, _ = pvq.popleft()
                op()
                if islast:
                    stage_runs.pop(key)(cur_idx)

        def pop_norm(cur_idx):
            while normq:
                normq.popleft()(cur_idx)

        # ---- main pipeline over units ----
        for idx, (sb, hp) in enumerate(units):
            if hp == 0:
                cat_tiles[sb] = cpool.tile([128, c.JC * c.S_BLK], DT,
                                           tag="cat", name=f"catT{sb}")
            if sb == 0 and hp == 3 and c.NSB > 2:
                # xq2 load here: xq0's ring slot is free (all qT(0) MMs were
                # emitted by the end of unit (0,2))
                xq_tiles[2] = load_x(xqpool, xqT, 2, c.S_BLK, "xq2")
            if sb == 0 and hp == 2 and c.NSB > 1:
                qT_tiles[1] = qpool.tile([128, c.JC * c.S_BLK], DT,
                                         tag="qT", name="qT1")
                q1 = emit_qT_mms(1, xq_tiles[1], qT_tiles[1])
                fillers[idx] += q1[:16]
                fillers[min(idx + 1, len(units) - 1)] += q1[16:]
            if sb >= 1 and hp == 0 and sb + 1 < c.NSB:
                # steady state: xq(sb+2) prefetch + qT(sb+1) fillers spread
                # over all four units of this sb
                if sb + 2 < c.NSB and sb + 2 not in xq_tiles:
                    def pfq(sb=sb):
                        xq_tiles[sb + 2] = load_x(xqpool, xqT, sb + 2,
                                                  c.S_BLK, f"xq{sb + 2}")
                    fillers[idx].append(pfq)
                qT_tiles[sb + 1] = qpool.tile([128, c.JC * c.S_BLK], DT,
                                              tag="qT", name=f"qT{sb + 1}")
                qops = emit_qT_mms(sb + 1, xq_tiles[sb + 1], qT_tiles[sb + 1])
                for j in range(4):
                    fillers[min(idx + j, len(units) - 1)] += qops[j * 8:(j + 1) * 8]
            catT = cat_tiles[sb]
            qT = qT_tiles[sb]
            key = (sb, hp)
            pv_state = {}
            stage_runs[key] = (
                lambda cur_idx, sb=sb, hp=hp, catT=catT, pv_state=pv_state:
                emit_stage(sb, hp, catT, pv_state, cur_idx))
            flist = fillers[idx]
            fpos = 0
            for th in range(c.TCn // 2):
                pop_norm(idx)
                for u in range(2):
                    t = 2 * th + u
                    # deep hold while sb-0's v' fillers are in flight, then
                    # taper back to the steady lag (avoids a pop burst)
                    if sb == 0 and hp <= 1:
                        lag = PVLAG0
                    elif idx == len(units) - 1:
                        # final unit: taper so its PV drains in-loop without
                        # a pop burst at the start (short tail)
                        lag = max(6, PVLAG0 - 2 * t)
                    else:
                        lag = PVLAG
                    kcol = slice(hp * c.S + t * 128, hp * c.S + (t + 1) * 128)
                    qcol = slice(hp * c.S_BLK, (hp + 1) * c.S_BLK)
                    # pops BEFORE the es alloc: the exp-slot ring may reuse a
                    # slot whose readers are exactly these pv pops. Backstop:
                    # if pops are v-gated and the queue nears the es-ring
                    # capacity, pull fillers forward to advance the v copies.
                    while len(pvq) >= 40 and fpos < len(flist):
                        flist[fpos]()
                        fpos += 1
                    pop_pv(len(pvq) - lag, idx)
                    # One 2-bank psum tile [A(512) | B(512)] per t-chunk:
                    # K=64 row-tiled pair (head A rows 0-63 tile (0,0), head B
                    # rows 64-127 tile (64,0)) in different banks; ONE exp
                    # covers both heads so the pair stays adjacent/concurrent.
                    ps2 = pspool.tile([128, 2 * c.S_BLK], f32, tag="ps",
                                      name=f"ps2_{sb}_{hp}_{t}")
                    nc.tensor.matmul(
                        ps2[:, 0:c.S_BLK],
                        kT_sb[0:64, kcol], qT[0:64, qcol],
                        start=True, stop=True)
                    nc.tensor.matmul(
                        ps2[:, c.S_BLK:2 * c.S_BLK],
                        kT_sb[64:128, kcol], qT[64:128, qcol],
                        start=True, stop=True)
                    es_t = epool.tile([128, 2 * c.S_BLK], EXPDT, tag="exp",
                                      name=f"es{sb}_{hp}_{t}")
                    nc.scalar.activation(
                        es_t[:], ps2[:], mybir.ActivationFunctionType.Exp,
                        scale=SCALE)
                    pvq.extend(make_pv_ops(key, sb, hp, t, es_t, pv_state))
                    want = (len(flist) * (t + 1)) // c.TCn
                    while fpos < want:
                        flist[fpos]()
                        fpos += 1
            # any fillers appended after pacing window closed
            while fpos < len(flist):
                flist[fpos]()
                fpos += 1
        # ---- drain ----
        pop_pv(len(pvq), len(units) - 1)
        pop_norm(len(units) - 1)
        for op in pending_tail:
            op()

    nc.compile()
    return nc


def shard_inputs(inputs: dict, cfg: Cfg, DT=mybir.dt.bfloat16):
    """Full inputs -> list of 8 per-core in_maps (numpy)."""
    npdt = DT_NP[DT]
    q, k, v = inputs["queries"], inputs["keys"], inputs["values"]
    Wq, Wk, Wv = inputs["Wq"], inputs["Wk"], inputs["Wv"]
    Wout = inputs["Wout"]
    B = q.shape[0]
    maps = []
    WoutT = np.ascontiguousarray(Wout.T)  # [i, o]
    for core in range(2 * B):
        b, half = divmod(core, 2)
        hs = slice(half * cfg.HL, (half + 1) * cfg.HL)
        i0 = half * cfg.JW
        maps.append({
            "xqT": np.ascontiguousarray(q[b].T).astype(npdt),
            "xkT": np.ascontiguousarray(k[b].T).astype(npdt),
            "xvT": np.ascontiguousarray(v[b].T).astype(npdt),
            "wq": np.ascontiguousarray(
                Wq[hs].transpose(1, 0, 2).reshape(cfg.D, cfg.JW)).astype(npdt),
            "wk": np.ascontiguousarray(
                Wk[hs].transpose(1, 0, 2).reshape(cfg.D, cfg.JW)).astype(npdt),
            "wv": np.ascontiguousarray(
                Wv[hs].transpose(1, 0, 2).reshape(cfg.D, cfg.JW)).astype(npdt),
            "woutT": np.ascontiguousarray(WoutT[i0:i0 + cfg.JW]).astype(npdt),
        })
    return maps


def gather_outputs(results, inputs):
    bout = inputs["bout"]
    B = inputs["queries"].shape[0]
    outs = []
    for b in range(B):
        outs.append(results[2 * b]["out"] + results[2 * b + 1]["out"] + bout)
    return np.stack(outs).astype(np.float32)


def percore_reference(in_map: dict, cfg: Cfg):
    """Numpy reference of what one core should produce (fp32 math)."""
    c = cfg
    xq = in_map["xqT"].astype(np.float32).T   # [S, D]
    xk = in_map["xkT"].astype(np.float32).T
    xv = in_map["xvT"].astype(np.float32).T
    wq = in_map["wq"].astype(np.float32)      # [D, JW]
    wk = in_map["wk"].astype(np.float32)
    wv = in_map["wv"].astype(np.float32)
    wo = in_map["woutT"].astype(np.float32)   # [JW, D]
    q = xq @ wq                               # [S, JW]
    k = xk @ wk
    v = xv @ wv
    cat = np.zeros((c.S, c.JW), dtype=np.float32)
    for h in range(c.HL):
        sl = slice(h * c.DK, (h + 1) * c.DK)
        s = (q[:, sl] @ k[:, sl].T) / np.sqrt(c.DK)
        e = np.exp(s)
        p = e / e.sum(axis=1, keepdims=True)
        cat[:, sl] = p @ v[:, sl]
    return cat @ wo

# ----------------------------------------------------------------------------
# Self-contained entry point: kernel(**inputs) -> full [B, S, D] output.
# ----------------------------------------------------------------------------
_NC_CACHE = {}


def _get_nc():
    key = "attn"
    if key not in _NC_CACHE:
        _NC_CACHE[key] = build_nc(Cfg(), mybir.dt.bfloat16, num_devices=8)
    return _NC_CACHE[key]


def kernel(**inputs):
    """Full (unsharded) inputs -> full [4, 2048, 1024] float32 output.

    Shards across the 8 NeuronCores as (batch x head-half), runs the Bass
    kernel SPMD, and gathers: out[b] = partial(core 2b) + partial(core 2b+1)
    + bias (row-sharded fc_out -> partial-sum reduction at gather time).
    """
    from concourse.bass_utils import run_bass_kernel_spmd

    inputs = {k: np.asarray(v) for k, v in inputs.items()}
    cfg = Cfg()
    nc = _get_nc()
    maps = shard_inputs(inputs, cfg, mybir.dt.bfloat16)
    res = run_bass_kernel_spmd(nc, maps, core_ids=list(range(8)), trace=False)
    return gather_outputs(res.results, inputs)

